# revision 24
# baseline (speedup 1.0000x reference)
"""Bilinear causal attention (nn_Attention_34772055228779) on 8 trn2 cores.

reference:
  scores[i,k] = x[i] @ W_bi[k] @ x[i]          [512, 512]
  attn = softmax(scores + causal_mask, axis=1)
  out  = (attn @ x) @ W_out.T                  [512, 512]

Device strategy (tensor-parallel over score columns, per sharding hint):
  core m holds the k-interleaved shard W_bi[m::8] (64 local columns).

  Only the symmetric part of W_bi[k] contributes to x^T W x, so the host
  packs U'_k = triu(W_k + W_k^T, 1) + diag(W_k)  (exact identity:
  x^T U' x = x^T W x).  U' is upper-triangular, so the d-row-block dt only
  has nonzeros in columns e >= 128*dt: the four matmul rhs spans are
  512/384/256/128 instead of 4x512 (37.5%% less PE work), and the packed
  fp16 stream is 320 KiB/k = 20 MiB/core instead of 64 MiB fp32.

  stage A (_build_v2): for each local k: Y_k = X16 @ U'16_k (fp16
           matmuls, fp32 PSUM, lhsT = X^T resident, 7 PSUM banks),
           scores[:, k] = rowsum(Y_k * X) -- mostly one fused DVE
           scalar_tensor_tensor per row-tile; 7 of every 40 tiles are
           routed [ACT copy f16 -> Pool mult -> ACT accum] to keep DVE
           just under the PE.  Causally dead row-tiles are skipped
           (SPMD-uniform bound with the k-interleaved sharding).
  Startup: first weight-pair DMAs split in half; x/ident residents ride
           the Pool SWDGE queue so the SP queue stays clear for the
           weight stream.
  Gather:  score columns are gathered to DRAM in 4 chunks as their
           stts complete; the final chunk is only 4 columns, so the
           AllToAll launches almost immediately after the last stt.
  tail:    masked softmax rows (DVE sub/max + ACT exp with fused denom
           accum; Exp table pre-warmed at body start), A^T via fp16 PE
           transposes into PSUM (saves ~8 us vs xbar DMA transposes),
           O^T = X^T A^T, Y = O @ W_out^T, per-partition 1/den folded
           into the final copy, DMA 64 rows out.
  host:    concatenates the 8 row blocks.

Measured (this hardware, single-NEFF dynamic-trip-count slope, stubbed
collective): 130-132 us/iter vs 147-150 us/iter for the previous
kernel; timeline-sim (cost-model) totals 113.6 vs 143.4 us.

Tried and rejected (measured, do not repeat blindly):
  - W-stationary stage A w/ exact causal streaming (_build_v3): PE busy
    drops 91->84 us but the e-block-sum vector work + longer per-chunk
    dependency chains add ~25 us of pipeline stalls (sim 136 vs 113.6).
  - tail consts / xtp on the ACT hwdge queue: ACT SEQ spends 667 ns
    issuing each DMA and delays the routed-path copies (HW: +4 us).
  - weight stream alternating SP/Pool queues: Pool SWDGE descriptor gen
    runs on the Pool engine (sim: +2..13 us).
  - routing more than 7/40 stts off DVE: the 3-op routed chain holds
    yp PSUM banks longer; bank pressure beats engine balance.
  - fp8 weights (score noise ~0.8 logits vs ~0.01 budget), lhsT-sharing
    matmul order (HW: no effect), 2D sharding (doubles weight DMA).
"""
import numpy as np

N_CTX = 512
D = 512
NCORES = 8
KSH = N_CTX // NCORES      # 64 score columns per core
RSH = N_CTX // NCORES      # 64 output rows per core
NEG_INF = -1e30
STAGE_A = "causal"   # "causal" skips fully-masked row-tiles (k-interleaved)

# upper-triangular pack: per dt row-block, columns [128*dt, 512)
SPANS = [512, 384, 256, 128]
OFFS = [0, 512, 896, 1152]          # column offset of block dt in the pack
PACKW = 1280                         # total packed width per partition

_nc_cache = None

# best Phase-1 configuration (sim-tuned)
V2_KW = dict(route_n=3, route_grp=40, route_slots=(3, 14, 25, 31, 9, 20, 36),
             xq_act="pool", gather4=True, sfull_act=True, warm_exp=True,
             wbufs=6, ppa_bufs=7)


def _build(timing_loop=0, use_collective=True, num_devices=NCORES,
           stage_a="causal", wbufs=4, stt_split=True, softmax_fused=False,
           gather_3d=True, debug_scores=False, route_red="act"):
    # NOTE: softmax_fused=True (tensor_tensor_reduce min) compiles but
    # crashes the exec unit on real TRN2 hardware -- keep it off.
    """Build the Bass module.

    timing_loop=R>0 wraps the whole per-core body in a hardware For_i loop
    (R iterations) for slope timing; collectives can't sit in control flow,
    so timing variants pass use_collective=False (the gather DMA then reads
    the pre-collective buffer -- wrong data, identical shapes/costs).
    """
    import concourse.mybir as mybir
    import concourse.tile as tile
    from concourse import bacc

    f32 = mybir.dt.float32
    f16 = mybir.dt.float16
    Alu = mybir.AluOpType
    Act = mybir.ActivationFunctionType

    nc = bacc.Bacc(
        "TRN2", target_bir_lowering=False, debug=False,
        enable_asserts=False, num_devices=num_devices,
    )

    # x row-major packed [p, nt, d] in f32 and f16: one DMA each
    x_t = nc.dram_tensor("x", [128, 4, D], f32, kind="ExternalInput").ap()
    x16_t = nc.dram_tensor("x16", [128, 4, D], f16, kind="ExternalInput").ap()
    # tail constants packed [p, 8, e] f16: [:,0:4] = column-permuted X rows
    # (k-interleaved layout) for attn @ X, [:,4:8] = W_out^T blocks
    xpwo_t = nc.dram_tensor("xpwo", [128, 8, D], f16,
                            kind="ExternalInput").ap()
    # X^T packed [p, dt, n]: one DMA loads all four lhsT d-blocks
    xtp_t = nc.dram_tensor("xtp", [128, 4, N_CTX], f16,
                           kind="ExternalInput").ap()
    # W pairs: [j] holds packed U' for columns kk=j and kk=63-j
    wbi_t = nc.dram_tensor("wbi", [KSH // 2, 128, 2 * PACKW], f16,
                           kind="ExternalInput").ap()
    # negated additive mask: 0 where allowed, +1e30 where causally masked
    mask_t = nc.dram_tensor("mask", [RSH, N_CTX], f32, kind="ExternalInput").ap()
    niter_t = (nc.dram_tensor("niter", [1, 1], mybir.dt.int32,
                              kind="ExternalInput").ap()
               if timing_loop == -1 else None)
    out_t = nc.dram_tensor("out", [RSH, D], f32, kind="ExternalOutput").ap()
    dbg_t = (nc.dram_tensor("dbg", [128, 4 * KSH], f32,
                            kind="ExternalOutput").ap()
             if debug_scores else None)

    with tile.TileContext(nc) as tc:
        with (
            tc.tile_pool(name="const", bufs=1) as cpool,
            tc.tile_pool(name="tailc", bufs=2) as tcpool,
            tc.tile_pool(name="wstream", bufs=wbufs) as wpool,
            tc.tile_pool(name="scratch", bufs=3) as spool,
            tc.tile_pool(name="scratch2", bufs=3) as spool2,
            tc.tile_pool(name="small", bufs=1) as mpool,
            tc.tile_pool(name="psA", bufs=6, space="PSUM") as ppA,
            tc.tile_pool(name="psB", bufs=2, space="PSUM") as ppB,
            tc.tile_pool(name="dram", bufs=1, space="DRAM") as dpool,
        ):
            # ---- resident loads (outside any timing loop) -----------------
            # xt first (single packed DMA): the first matmul only needs
            # xt + wk0, so the x/x16 loads (needed ~2.6us later by the
            # first stt) are issued after the first wk DMA to cut the
            # startup serial chain.
            xtp_sb = cpool.tile([128, 4, N_CTX], f16, tag="xtp", name="xtp")
            nc.sync.dma_start(xtp_sb[:], xtp_t[:])
            xpk_sb = cpool.tile([128, 4, N_CTX], f32, tag="xpk", name="xpk")
            x16k_sb = cpool.tile([128, 4, N_CTX], f16, tag="x16k",
                                 name="x16k")

            def load_x_resident():
                nc.sync.dma_start(xpk_sb[:], x_t[:])
                nc.sync.dma_start(x16k_sb[:], x16_t[:])
            # single score accumulator tile, column nt*KSH + kk
            scores_sb = cpool.tile([128, 4 * KSH], f32, tag="sc", name="sc")
            # skipped (nt, kk) cells are never written; zero them so no
            # NaN bit-patterns survive into exp() past the additive mask
            nc.gpsimd.memset(scores_sb[:], 0.0)
            agin = dpool.tile([N_CTX, KSH], f32, tag="agin")
            agout = dpool.tile([N_CTX, KSH], f32, tag="agout")
            agin_v = agin[:].rearrange("(t p) k -> p t k", p=128)
            scores_v = scores_sb[:].rearrange("p (t k) -> p t k", t=4)

            def load_wk_pair(j):
                # one DMA covers both columns of the pair (j, 63-j)
                wk = wpool.tile([128, 2 * PACKW], f16, tag="wk", name="wk")
                nc.sync.dma_start(wk[:], wbi_t[j])
                return wk

            # stt engine split: only DVE can reduce straight from PSUM
            # (Pool has no PSUM access and TensorScalarPtr is not a legal
            # Pool opcode).  A share of tiles is therefore routed
            #   ACT:  yp (PSUM f32) -> y16 (SBUF f16)
            #   Pool: prod16 = y16 * x16          (TensorTensor, SBUF)
            #   ACT:  Copy(prod16) with accum_out -> scores column
            # Costs: DVE stt ~658 ns; ACT ~2x660 ns and Pool ~840 ns per
            # routed tile.  6 of every 20 tiles (spread, not consecutive,
            # so DVE never sits idle for long) puts DVE ~76us, ACT ~64us
            # and Pool ~40us, all under the ~90us PE stage-A floor.
            POOL_SLOTS = {3, 6, 9, 13, 16, 19}
            stt_state = {"i": 0}

            def emit_stt(yp, nt, kk):
                if stt_split:
                    use_dve = (stt_state["i"] % 20) not in POOL_SLOTS
                    stt_state["i"] += 1
                else:
                    use_dve = True
                col = nt * KSH + kk
                if use_dve:
                    scr = spool.tile([128, D], f32, tag="stt_out", name="scr")
                    nc.vector.scalar_tensor_tensor(
                        out=scr[:], in0=yp[:], scalar=1.0,
                        in1=xpk_sb[:, nt, :],
                        op0=Alu.mult, op1=Alu.mult,
                        accum_out=scores_sb[:, col:col + 1],
                    )
                else:
                    y16 = spool2.tile([128, D], f16, tag="y16", name="y16")
                    nc.scalar.copy(y16[:], yp[:])
                    prod = spool2.tile([128, D], f16, tag="prod", name="prod")
                    nc.gpsimd.tensor_tensor(
                        out=prod[:], in0=y16[:], in1=x16k_sb[:, nt, :],
                        op=Alu.mult)
                    if route_red == "dve":
                        nc.vector.tensor_reduce(
                            scores_sb[:, col:col + 1], prod[:],
                            axis=mybir.AxisListType.X, op=Alu.add)
                    else:
                        scr = spool2.tile([128, D], f16, tag="scr16",
                                          name="scr16")
                        nc.scalar.activation(
                            scr[:], prod[:], Act.Copy, bias=0.0, scale=1.0,
                            accum_out=scores_sb[:, col:col + 1])

            def stage_a_tri(load_tail_consts):
                # causal: with k-interleaved sharding (global k = 8*kk + m),
                # row-tiles nt < kk//16 are fully masked for column kk on
                # EVERY core, so the skip bound is SPMD-uniform.
                #
                # Column order pairs kk with 63-kk: every pair is exactly 5
                # kept row-tiles of PE work against 2 wk DMAs, so the DMA
                # stream never outpaces nor starves the PE (a plain
                # ascending order leaves PE idle behind DMA for the late,
                # 1-tile columns).
                for j in range(KSH // 2):
                    wk = load_wk_pair(j)
                    if j == 0:
                        # must precede the first stt in program order: the
                        # dependency tracker only orders reads after writes
                        # that were already emitted
                        load_x_resident()
                    if j == 10:
                        # late enough that the wk pair stream has built a
                        # surplus on the shared DMA engines; the constants
                        # still land ~70us before the tail reads them
                        load_tail_consts()
                    for half, kk in enumerate((j, KSH - 1 - j)):
                        base = half * PACKW
                        nt_lo = (kk // 16) if stage_a == "causal" else 0
                        for nt in range(nt_lo, 4):
                            yp = ppA.tile([128, D], f32, tag="yp", name="yp")
                            for dt in range(4):
                                span = SPANS[dt]
                                nc.tensor.matmul(
                                    yp[:, D - span:D],
                                    lhsT=xtp_sb[:, dt,
                                                nt * 128:(nt + 1) * 128],
                                    rhs=wk[:, base + OFFS[dt]:
                                           base + OFFS[dt] + span],
                                    start=(dt == 0),
                                    stop=(dt == 3),
                                    skip_group_check=True,
                                )
                            emit_stt(yp, nt, kk)
                    if j == 15 and gather_3d:
                        # columns {0..15, 48..63} are final: start their
                        # DRAM gather under the remaining compute.  On the
                        # Pool SWDGE queue so the wait on those columns'
                        # stts never blocks the SP weight-stream queue.
                        nc.gpsimd.dma_start(
                            agin_v[:, :, 0:16], scores_v[:, :, 0:16])
                        nc.gpsimd.dma_start(
                            agin_v[:, :, 48:64], scores_v[:, :, 48:64])

            def body():
                # tail constants, double-buffered (bufs=2) so the timing
                # loop's next iteration can re-load them without a
                # write-after-read stall against this iteration's tail
                tail_c = {}

                def load_tail_consts():
                    tail_c["xpwo"] = tcpool.tile(
                        [128, 8, N_CTX], f16, tag="xpwo", name="xpwo")
                    tail_c["mask"] = tcpool.tile(
                        [RSH, N_CTX], f32, tag="mask", name="mask")
                    nc.sync.dma_start(tail_c["xpwo"][:], xpwo_t[:])
                    nc.sync.dma_start(tail_c["mask"][:], mask_t[:])

                # ---- stage A: local score columns -------------------------
                stage_a_tri(load_tail_consts)
                xpwo_sb = tail_c["xpwo"]
                mask_sb = tail_c["mask"]

                # ---- AllToAll: shard columns -> shard rows ----------------
                # (columns {0..15, 48..63} were already gathered mid-stage-A)
                # Gather/scatter DMAs ride the Pool SWDGE queue, same as the
                # collective, keeping the SP queue free for the next
                # iteration's weight stream.
                if gather_3d:
                    nc.gpsimd.dma_start(
                        agin_v[:, :, 16:48], scores_v[:, :, 16:48])
                else:
                    for nt in range(4):
                        nc.gpsimd.dma_start(
                            agin[nt * 128:(nt + 1) * 128, :],
                            scores_sb[:, nt * KSH:(nt + 1) * KSH])
                if use_collective:
                    nc.gpsimd.collective_compute(
                        "AllToAll",
                        mybir.AluOpType.bypass,
                        replica_groups=[list(range(NCORES))],
                        ins=[agin[:].opt()],
                        outs=[agout[:].opt()],
                    )
                    coll_out = agout
                else:
                    coll_out = agin
                # rows of the full score matrix for this core: [64, 512]
                sfull = mpool.tile([RSH, N_CTX], f32, tag="sfull", name="sfull")
                nc.gpsimd.dma_start(
                    sfull[:].rearrange("i (r k) -> i r k", r=NCORES),
                    coll_out[:].rearrange("(r i) k -> i r k", r=NCORES),
                )

                # ---- masked softmax over the 64 rows ----------------------
                # fused mask+max: nsm = negmask - scores (so masked cells are
                # ~+1e30 and min(nsm) = -max of the allowed scores), then
                # exp(-nsm + bias) on ACT.  The 1/denominator is folded into
                # the final output copy as a per-partition ACT scale, keeping
                # the reciprocal off the critical path.
                nsm = mpool.tile([RSH, N_CTX], f32, tag="sm", name="sm")
                negm = mpool.tile([RSH, 1], f32, tag="negm", name="negm")
                esb = mpool.tile([RSH, N_CTX], f16, tag="esb", name="esb")
                den = mpool.tile([RSH, 1], f32, tag="den", name="den")
                if softmax_fused:
                    # nsm = negmask - s (masked cells ~ +1e30), negm =
                    # min(nsm) = -max over allowed, exp(-nsm + negm)
                    nc.vector.tensor_tensor_reduce(
                        out=nsm[:], in0=mask_sb[:], in1=sfull[:], scale=1.0,
                        scalar=float(-NEG_INF), op0=Alu.subtract, op1=Alu.min,
                        accum_out=negm[:])
                    nc.scalar.activation(
                        esb[:], nsm[:], Act.Exp, bias=negm[:], scale=-1.0,
                        accum_out=den[:])
                else:
                    # sm = s - negmask (masked cells ~ -1e30)
                    nc.vector.tensor_tensor(
                        out=nsm[:], in0=sfull[:], in1=mask_sb[:],
                        op=Alu.subtract)
                    nc.vector.reduce_max(
                        negm[:], nsm[:], axis=mybir.AxisListType.X,
                        negate=True)
                    nc.scalar.activation(
                        esb[:], nsm[:], Act.Exp, bias=negm[:], scale=1.0,
                        accum_out=den[:])
                rden = mpool.tile([RSH, 1], f32, tag="rden", name="rden")
                nc.vector.reciprocal(rden[:], den[:])

                # ---- A^T via xbar DMA transpose: [64, 512] -> 4x [128, 64]
                # (unnormalized fp16 exp weights; dispatched on the ACT
                # HWDGE queue so same-engine ordering after the exp makes
                # the chain wait-free)
                at_sb = []
                for kt in range(4):
                    at = mpool.tile([128, RSH], f16, tag=f"at{kt}",
                                    name=f"at{kt}")
                    nc.scalar.dma_start_transpose(
                        at[:], esb[:, kt * 128:(kt + 1) * 128])
                    at_sb.append(at)

                # ---- O^T = X^T @ A^T : [512(e), 64(i)] --------------------
                ot_sb = []
                for et in range(4):
                    op = ppB.tile([128, 512], f32, tag="tail", name="op")
                    for kt in range(4):
                        nc.tensor.matmul(
                            op[:, 0:RSH],
                            lhsT=xpwo_sb[:, kt, et * 128:(et + 1) * 128],
                            rhs=at_sb[kt][:],
                            start=(kt == 0),
                            stop=(kt == 3),
                        )
                    ot = mpool.tile([128, RSH], f16, tag=f"ot{et}",
                                    name=f"ot{et}")
                    nc.scalar.copy(ot[:], op[:, 0:RSH])
                    ot_sb.append(ot)

                # ---- Y = O @ W_out^T : [64(i), 512(f)] --------------------
                ypz = ppB.tile([128, 512], f32, tag="tail", name="ypz")
                for et in range(4):
                    nc.tensor.matmul(
                        ypz[0:RSH, :],
                        lhsT=ot_sb[et][:],
                        rhs=xpwo_sb[:, 4 + et, :],
                        start=(et == 0),
                        stop=(et == 3),
                    )
                # final copy normalizes the softmax: per-partition 1/den
                y_sb = mpool.tile([RSH, D], f32, tag="y_sb", name="y_sb")
                nc.scalar.mul(y_sb[:], ypz[0:RSH, :], rden[:])
                nc.scalar.dma_start(out_t[:], y_sb[:])
                if debug_scores:
                    nc.sync.dma_start(dbg_t[:], scores_sb[:])

            if timing_loop == -1:
                # dynamic trip count from the niter input: one NEFF serves
                # every loop length, so slope measurements compare runs of
                # the SAME executable (per-executable launch offsets cancel)
                tmp = nc.alloc_registers("niter_reg", mybir.ALL_ENGINES)
                nc.regs_load(tmp, niter_t[0:1, 0:1])
                nval = nc.snap(tmp, donate=True, min_val=0, max_val=1024)
                with tc.For_i(0, nval, 1):
                    body()
            elif timing_loop:
                with tc.For_i(0, timing_loop, 1):
                    body()
            else:
                body()

    nc.compile()
    return nc


def _build_v2(timing_loop=0, use_collective=True, num_devices=NCORES,
              wbufs=6, route_n=6, route_grp=20, pe_transpose=True,
              gather3=True, split_first=2, ppa_bufs=7, route_red="act",
              xq_act=True, gather4=True, sfull_act=True, warm_exp=True,
              route_slots=None, tailc_act=False, xtp_act=False,
              wk_alt=0, xtp_interleave=False, last_gather_act=False,
              tail_route=(), x16_stt=False):
    """Phase-1 rework of _build: 7-bank stage-A PSUM (tail reuses them),
    retuned stt routing (ACT copy -> Pool tt -> ACT accum), split first
    weight DMAs (startup latency), 3-chunk score gather, PE-transpose tail.
    """
    import concourse.mybir as mybir
    import concourse.tile as tile
    from concourse import bacc

    f32 = mybir.dt.float32
    f16 = mybir.dt.float16
    Alu = mybir.AluOpType
    Act = mybir.ActivationFunctionType

    nc = bacc.Bacc(
        "TRN2", target_bir_lowering=False, debug=False,
        enable_asserts=False, num_devices=num_devices,
    )

    x_t = nc.dram_tensor("x", [128, 4, D], f32, kind="ExternalInput").ap()
    x16_t = nc.dram_tensor("x16", [128, 4, D], f16, kind="ExternalInput").ap()
    xpwo_t = nc.dram_tensor("xpwo", [128, 8, D], f16,
                            kind="ExternalInput").ap()
    xtp_t = nc.dram_tensor("xtp", [128, 4, N_CTX], f16,
                           kind="ExternalInput").ap()
    wbi_t = nc.dram_tensor("wbi", [KSH // 2, 128, 2 * PACKW], f16,
                           kind="ExternalInput").ap()
    mask_t = nc.dram_tensor("mask", [RSH, N_CTX], f32,
                            kind="ExternalInput").ap()
    ident_t = nc.dram_tensor("ident", [64, 64], f16,
                             kind="ExternalInput").ap()
    niter_t = (nc.dram_tensor("niter", [1, 1], mybir.dt.int32,
                              kind="ExternalInput").ap()
               if timing_loop == -1 else None)
    out_t = nc.dram_tensor("out", [RSH, D], f32, kind="ExternalOutput").ap()

    with tile.TileContext(nc) as tc:
        with (
            tc.tile_pool(name="const", bufs=1) as cpool,
            tc.tile_pool(name="tailc", bufs=2) as tcpool,
            tc.tile_pool(name="wstream", bufs=wbufs) as wpool,
            tc.tile_pool(name="scratch", bufs=3) as spool,
            tc.tile_pool(name="scratch2", bufs=3) as spool2,
            tc.tile_pool(name="small", bufs=1) as mpool,
            tc.tile_pool(name="psA", bufs=ppa_bufs, space="PSUM") as ppA,
            tc.tile_pool(name="psB", bufs=1, space="PSUM") as ppB,
            tc.tile_pool(name="dram", bufs=1, space="DRAM") as dpool,
        ):
            # ---- resident loads ------------------------------------------
            # xtp rides the ACT queue in two halves so the SP queue opens
            # with the first weight DMA and the first matmul (needing only
            # xtp[:, 0]) starts ~1.5us earlier
            xtp_sb = cpool.tile([128, 4, N_CTX], f16, tag="xtp", name="xtp")
            if xtp_act:
                nc.scalar.dma_start(xtp_sb[:, 0:2, :], xtp_t[:, 0:2, :])
                nc.scalar.dma_start(xtp_sb[:, 2:4, :], xtp_t[:, 2:4, :])
            elif xtp_interleave:
                # only the dt 0/1 half ahead of the first weight DMA; the
                # dt 2/3 half is issued right after it (stage_a j==0)
                nc.sync.dma_start(xtp_sb[:, 0:2, :], xtp_t[:, 0:2, :])
            else:
                nc.sync.dma_start(xtp_sb[:], xtp_t[:])
            xpk_sb = cpool.tile([128, 4, N_CTX], f32, tag="xpk", name="xpk")
            x16k_sb = cpool.tile([128, 4, N_CTX], f16, tag="x16k",
                                 name="x16k")
            ident_sb = cpool.tile([64, 64], f16, tag="ident", name="ident")

            def load_x_resident():
                # ACT hwdge queue: keeps the SP queue clear for the wk
                # stream (x loads there stalled PE ~5us at startup)
                eng = {"act": nc.scalar, "pool": nc.gpsimd,
                       "sp": nc.sync}[xq_act if isinstance(xq_act, str)
                                      else ("act" if xq_act else "sp")]
                eng.dma_start(xpk_sb[:], x_t[:])
                eng.dma_start(x16k_sb[:], x16_t[:])
                eng.dma_start(ident_sb[:], ident_t[:])
            scores_sb = cpool.tile([128, 4 * KSH], f32, tag="sc", name="sc")
            nc.gpsimd.memset(scores_sb[:], 0.0)
            agin = dpool.tile([N_CTX, KSH], f32, tag="agin")
            agout = dpool.tile([N_CTX, KSH], f32, tag="agout")
            agin_v = agin[:].rearrange("(t p) k -> p t k", p=128)
            scores_v = scores_sb[:].rearrange("p (t k) -> p t k", t=4)

            def load_wk_pair(j, split=1):
                wk = wpool.tile([128, 2 * PACKW], f16, tag="wk", name="wk")
                eng = nc.gpsimd if (wk_alt and j % wk_alt == wk_alt - 1) \
                    else nc.sync
                if split == 1:
                    eng.dma_start(wk[:], wbi_t[j])
                else:
                    eng.dma_start(wk[:, 0:PACKW], wbi_t[j][:, 0:PACKW])
                    eng.dma_start(wk[:, PACKW:], wbi_t[j][:, PACKW:])
                return wk

            # stt: DVE direct, or routed [ACT copy f16 -> Pool tt -> ACT
            # accum].  route_n of every route_grp tiles take the routed path.
            if route_slots is not None:
                ROUTE_SLOTS = set(route_slots)
            else:
                ROUTE_SLOTS = set()
                if route_n:
                    step = route_grp / route_n
                    ROUTE_SLOTS = {int(step * i + step / 2)
                                   for i in range(route_n)}
            stt_state = {"i": 0}

            def emit_stt(yp, nt, kk):
                i = stt_state["i"]
                use_dve = ((i % route_grp) not in ROUTE_SLOTS
                           and i not in tail_route)
                stt_state["i"] += 1
                col = nt * KSH + kk
                if use_dve:
                    scr = spool.tile([128, D], f32, tag="stt_out", name="scr")
                    nc.vector.scalar_tensor_tensor(
                        out=scr[:], in0=yp[:], scalar=1.0,
                        in1=(x16k_sb if x16_stt else xpk_sb)[:, nt, :],
                        op0=Alu.mult, op1=Alu.mult,
                        accum_out=scores_sb[:, col:col + 1],
                    )
                else:
                    y16 = spool2.tile([128, D], f16, tag="y16", name="y16")
                    nc.scalar.copy(y16[:], yp[:])
                    prod = spool2.tile([128, D], f16, tag="prod", name="prod")
                    nc.gpsimd.tensor_tensor(
                        out=prod[:], in0=y16[:], in1=x16k_sb[:, nt, :],
                        op=Alu.mult)
                    if route_red == "dve":
                        nc.vector.tensor_reduce(
                            scores_sb[:, col:col + 1], prod[:],
                            axis=mybir.AxisListType.X, op=Alu.add)
                    else:
                        scr = spool2.tile([128, D], f16, tag="scr16",
                                          name="scr16")
                        nc.scalar.activation(
                            scr[:], prod[:], Act.Copy, bias=0.0, scale=1.0,
                            accum_out=scores_sb[:, col:col + 1])

            def stage_a(load_tail_consts):
                for j in range(KSH // 2):
                    wk = load_wk_pair(j, split=(2 if j < split_first else 1))
                    if j == 0:
                        if xtp_interleave and not xtp_act:
                            nc.sync.dma_start(xtp_sb[:, 2:4, :],
                                              xtp_t[:, 2:4, :])
                        load_x_resident()
                    if j == 10:
                        load_tail_consts()
                    for half, kk in enumerate((j, KSH - 1 - j)):
                        base = half * PACKW
                        nt_lo = kk // 16
                        for nt in range(nt_lo, 4):
                            yp = ppA.tile([128, D], f32, tag="yp", name="yp")
                            for dt in range(4):
                                span = SPANS[dt]
                                nc.tensor.matmul(
                                    yp[:, D - span:D],
                                    lhsT=xtp_sb[:, dt,
                                                nt * 128:(nt + 1) * 128],
                                    rhs=wk[:, base + OFFS[dt]:
                                           base + OFFS[dt] + span],
                                    start=(dt == 0),
                                    stop=(dt == 3),
                                    skip_group_check=True,
                                )
                            emit_stt(yp, nt, kk)
                    if gather3 and j == 15:
                        # columns {0..15, 48..63} final: gather now
                        nc.gpsimd.dma_start(
                            agin_v[:, :, 0:16], scores_v[:, :, 0:16])
                        nc.gpsimd.dma_start(
                            agin_v[:, :, 48:64], scores_v[:, :, 48:64])
                    if gather3 and j == 23:
                        # columns {16..23, 40..47} final
                        nc.gpsimd.dma_start(
                            agin_v[:, :, 16:24], scores_v[:, :, 16:24])
                        nc.gpsimd.dma_start(
                            agin_v[:, :, 40:48], scores_v[:, :, 40:48])
                    if gather3 and gather4 and j == 29:
                        # columns {24..29, 34..39} final
                        nc.gpsimd.dma_start(
                            agin_v[:, :, 24:30], scores_v[:, :, 24:30])
                        nc.gpsimd.dma_start(
                            agin_v[:, :, 34:40], scores_v[:, :, 34:40])

            def body():
                if warm_exp:
                    # warm the ACT Exp table while the PE pipeline fills:
                    # the implicit table load (~1.3us) otherwise lands
                    # right before the tail exp, on the critical path
                    warm = mpool.tile([1, 1], f32, tag="warm", name="warm")
                    nc.scalar.activation(warm[:], warm[:], Act.Exp,
                                         bias=0.0, scale=0.0)
                tail_c = {}

                def load_tail_consts():
                    # ACT hwdge queue: a 1.1 MiB load on the SP queue here
                    # would stall the weight stream behind it
                    teng = nc.scalar if tailc_act else nc.sync
                    tail_c["xpwo"] = tcpool.tile(
                        [128, 8, N_CTX], f16, tag="xpwo", name="xpwo")
                    tail_c["mask"] = tcpool.tile(
                        [RSH, N_CTX], f32, tag="mask", name="mask")
                    teng.dma_start(tail_c["xpwo"][:], xpwo_t[:])
                    teng.dma_start(tail_c["mask"][:], mask_t[:])

                # ---- stage A ---------------------------------------------
                stage_a(load_tail_consts)
                xpwo_sb = tail_c["xpwo"]
                mask_sb = tail_c["mask"]

                # ---- gather remaining columns + AllToAll ------------------
                if gather3 and gather4:
                    # final 4-column chunk on the ACT hwdge queue: skips
                    # the Pool SWDGE descriptor-gen on the critical path
                    (nc.scalar if last_gather_act else nc.gpsimd).dma_start(
                        agin_v[:, :, 30:34], scores_v[:, :, 30:34])
                elif gather3:
                    nc.gpsimd.dma_start(
                        agin_v[:, :, 24:40], scores_v[:, :, 24:40])
                else:
                    nc.gpsimd.dma_start(agin_v[:], scores_v[:])
                if use_collective:
                    nc.gpsimd.collective_compute(
                        "AllToAll",
                        mybir.AluOpType.bypass,
                        replica_groups=[list(range(NCORES))],
                        ins=[agin[:].opt()],
                        outs=[agout[:].opt()],
                    )
                    coll_out = agout
                else:
                    coll_out = agin
                sfull = mpool.tile([RSH, N_CTX], f32, tag="sfull",
                                   name="sfull")
                (nc.scalar if sfull_act else nc.gpsimd).dma_start(
                    sfull[:].rearrange("i (r k) -> i r k", r=NCORES),
                    coll_out[:].rearrange("(r i) k -> i r k", r=NCORES),
                )

                # ---- masked softmax over the 64 rows ----------------------
                nsm = mpool.tile([RSH, N_CTX], f32, tag="sm", name="sm")
                negm = mpool.tile([RSH, 1], f32, tag="negm", name="negm")
                esb = mpool.tile([RSH, N_CTX], f16, tag="esb", name="esb")
                den = mpool.tile([RSH, 1], f32, tag="den", name="den")
                nc.vector.tensor_tensor(
                    out=nsm[:], in0=sfull[:], in1=mask_sb[:],
                    op=Alu.subtract)
                nc.vector.reduce_max(
                    negm[:], nsm[:], axis=mybir.AxisListType.X,
                    negate=True)
                nc.scalar.activation(
                    esb[:], nsm[:], Act.Exp, bias=negm[:], scale=1.0,
                    accum_out=den[:])
                rden = mpool.tile([RSH, 1], f32, tag="rden", name="rden")
                nc.vector.reciprocal(rden[:], den[:])

                # ---- A^T: [64, 512] -> 4x [128, 64] -----------------------
                at_sb = []
                if pe_transpose:
                    pt = ppB.tile([128, 4, RSH], f16, tag="pt", name="pt")
                    for kt in range(4):
                        nc.tensor.transpose(
                            pt[:, kt, :], esb[:, kt * 128:(kt + 1) * 128],
                            ident_sb[:])
                    for kt in range(4):
                        at = mpool.tile([128, RSH], f16, tag=f"at{kt}",
                                        name=f"at{kt}")
                        nc.scalar.copy(at[:], pt[:, kt, :])
                        at_sb.append(at)
                else:
                    for kt in range(4):
                        at = mpool.tile([128, RSH], f16, tag=f"at{kt}",
                                        name=f"at{kt}")
                        nc.scalar.dma_start_transpose(
                            at[:], esb[:, kt * 128:(kt + 1) * 128])
                        at_sb.append(at)

                # ---- O^T = X^T @ A^T -------------------------------------
                ot_sb = []
                for et in range(4):
                    op = ppA.tile([128, RSH], f32, tag="yp", name="op")
                    for kt in range(4):
                        nc.tensor.matmul(
                            op[:, 0:RSH],
                            lhsT=xpwo_sb[:, kt, et * 128:(et + 1) * 128],
                            rhs=at_sb[kt][:],
                            start=(kt == 0),
                            stop=(kt == 3),
                        )
                    ot = mpool.tile([128, RSH], f16, tag=f"ot{et}",
                                    name=f"ot{et}")
                    nc.scalar.copy(ot[:], op[:, 0:RSH])
                    ot_sb.append(ot)

                # ---- Y = O @ W_out^T -------------------------------------
                ypz = ppB.tile([128, 512], f32, tag="pt", name="ypz")
                for et in range(4):
                    nc.tensor.matmul(
                        ypz[0:RSH, :],
                        lhsT=ot_sb[et][:],
                        rhs=xpwo_sb[:, 4 + et, :],
                        start=(et == 0),
                        stop=(et == 3),
                    )
                y_sb = mpool.tile([RSH, D], f32, tag="y_sb", name="y_sb")
                nc.scalar.mul(y_sb[:], ypz[0:RSH, :], rden[:])
                nc.scalar.dma_start(out_t[:], y_sb[:])

            if timing_loop == -1:
                tmp = nc.alloc_registers("niter_reg", mybir.ALL_ENGINES)
                nc.regs_load(tmp, niter_t[0:1, 0:1])
                nval = nc.snap(tmp, donate=True, min_val=0, max_val=1024)
                with tc.For_i(0, nval, 1):
                    body()
            elif timing_loop:
                with tc.For_i(0, timing_loop, 1):
                    body()
            else:
                body()

    nc.compile()
    return nc


def _build_v3(timing_loop=0, use_collective=True, num_devices=NCORES,
              wbufs=6, zbufs=3, n_direct=10, n_pool=20, warm_exp=True,
              st_groups=2, lag_pairs=1, sum_pool=5):
    """Phase-2: W-stationary stage A with exact causal streaming.

    Per local column kk (global k = 8*kk + m), only rows i >= 8*kk are
    computed (nk = 512 - 8*kk, SPMD-uniform).  The U' pack per kk is 10
    [128,128] blocks (upper block-triangle); each is the matmul lhsT and
    X^T streams as rhs: Z[et][e,i] = sum_d U'[d,e] x[i,d], nk columns per
    matmul -> 10*nk PE cycles/kk vs 1280*ceil(nk/128) row-tile cycles in
    the row-stationary layout (166k vs 205k cycles/core).

    scores^T assembly: P = Z (.) X^T reduced over e via an accumulating
    "ones-column" matmul: lhsT = ones32[kk%32] (ones in column kk%32)
    lands the row sum at PSUM partition kk of a [64, 512] score bank, nk
    cycles per kk.  The e-block sum over 4 blocks happens on DVE f16 adds
    beforehand; the PSUM->f16 copy runs on ACT (or the (.) on Pool) for a
    tunable share of columns to balance the three vector engines.
    """
    import concourse.mybir as mybir
    import concourse.tile as tile
    from concourse import bacc

    f32 = mybir.dt.float32
    f16 = mybir.dt.float16
    Alu = mybir.AluOpType
    Act = mybir.ActivationFunctionType

    nc = bacc.Bacc(
        "TRN2", target_bir_lowering=False, debug=False,
        enable_asserts=False, num_devices=num_devices,
    )

    xpwo_t = nc.dram_tensor("xpwo", [128, 8, D], f16,
                            kind="ExternalInput").ap()
    xtp_t = nc.dram_tensor("xtp", [128, 4, N_CTX], f16,
                           kind="ExternalInput").ap()
    wbi_t = nc.dram_tensor("wbi3", [KSH // 2, 128, 2 * PACKW], f16,
                           kind="ExternalInput").ap()
    mask_t = nc.dram_tensor("mask", [RSH, N_CTX], f32,
                            kind="ExternalInput").ap()
    ident_t = nc.dram_tensor("ident", [64, 64], f16,
                             kind="ExternalInput").ap()
    ident32_t = nc.dram_tensor("ident32", [64, 64], f32,
                               kind="ExternalInput").ap()
    ones32_t = nc.dram_tensor("ones32", [128, 32, 32], f16,
                              kind="ExternalInput").ap()
    niter_t = (nc.dram_tensor("niter", [1, 1], mybir.dt.int32,
                              kind="ExternalInput").ap()
               if timing_loop == -1 else None)
    out_t = nc.dram_tensor("out", [RSH, D], f32, kind="ExternalOutput").ap()

    with tile.TileContext(nc) as tc:
        with (
            tc.tile_pool(name="const", bufs=1) as cpool,
            tc.tile_pool(name="tailc", bufs=2) as tcpool,
            tc.tile_pool(name="wstream", bufs=wbufs) as wpool,
            tc.tile_pool(name="scratch", bufs=4) as spool,
            tc.tile_pool(name="scratch2", bufs=4) as spool2,
            tc.tile_pool(name="small", bufs=1) as mpool,
            tc.tile_pool(name="psZ", bufs=zbufs, space="PSUM") as ppZ,
            tc.tile_pool(name="psS", bufs=2, space="PSUM") as ppS,
            tc.tile_pool(name="dram", bufs=1, space="DRAM") as dpool,
        ):
            # ---- residents ------------------------------------------------
            xtp_sb = cpool.tile([128, 4, N_CTX], f16, tag="xtp", name="xtp")
            nc.sync.dma_start(xtp_sb[:], xtp_t[:])
            ones_sb = cpool.tile([128, 32, 32], f16, tag="o32", name="o32")
            ident_sb = cpool.tile([64, 64], f16, tag="ident", name="ident")
            id32_sb = cpool.tile([64, 64], f32, tag="id32", name="id32")

            def load_consts():
                nc.gpsimd.dma_start(ones_sb[:], ones32_t[:])
                nc.gpsimd.dma_start(ident_sb[:], ident_t[:])
                nc.gpsimd.dma_start(id32_sb[:], ident32_t[:])
            scores_sb = cpool.tile([128, 4 * KSH], f32, tag="sc", name="sc")
            zlhs_sb = cpool.tile([128, 32], f16, tag="zl", name="zl")
            nc.gpsimd.memset(zlhs_sb[:], 0.0)
            agin = dpool.tile([N_CTX, KSH], f32, tag="agin")
            agout = dpool.tile([N_CTX, KSH], f32, tag="agout")
            agin_v = agin[:].rearrange("(t p) k -> p t k", p=128)
            scores_v = scores_sb[:].rearrange("p (t k) -> p t k", t=4)

            def load_wk_pair(j, split=1):
                wk = wpool.tile([128, 2 * PACKW], f16, tag="wk", name="wk")
                if split == 1:
                    nc.sync.dma_start(wk[:], wbi_t[j])
                else:
                    nc.sync.dma_start(wk[:, 0:PACKW], wbi_t[j][:, 0:PACKW])
                    nc.sync.dma_start(wk[:, PACKW:], wbi_t[j][:, PACKW:])
                return wk

            # (et, dt) block order within a half-pack: et asc, dt asc
            BLK = {}
            _i = 0
            for et in range(4):
                for dt in range(et + 1):
                    BLK[(et, dt)] = _i
                    _i += 1

            # vector-chain routing per kk: "direct" (DVE stt from PSUM),
            # "actdve" (ACT copy f16 -> DVE (.) + adds), "actpool" (ACT
            # copy -> Pool (.) -> DVE adds)
            def route_of(kk):
                r = (kk * 7) % 16
                if r < n_direct:
                    return "direct"
                if r < n_direct + n_pool:
                    return "actpool"
                return "actdve"

            def body(st_banks):
                if warm_exp:
                    warm = mpool.tile([1, 1], f32, tag="warm", name="warm")
                    nc.scalar.activation(warm[:], warm[:], Act.Exp,
                                         bias=0.0, scale=0.0)
                tail_c = {}

                def load_tail_consts():
                    tail_c["xpwo"] = tcpool.tile(
                        [128, 8, N_CTX], f16, tag="xpwo", name="xpwo")
                    tail_c["mask"] = tcpool.tile(
                        [RSH, N_CTX], f32, tag="mask", name="mask")
                    nc.scalar.dma_start(tail_c["xpwo"][:], xpwo_t[:])
                    nc.scalar.dma_start(tail_c["mask"][:], mask_t[:])

                # ---- stage A: Z = U'^T-blocks @ X^T, scores^T rows -------
                # Z is produced in n-chunks of <=256 columns: one
                # [128, 4(et), 256] f32 tile = 2 PSUM banks, zbufs in
                # flight, so the vector chain for chunk c runs while the
                # PE fills chunk c+1/c+2 (one-pair emission lag).
                # st banks are zero-filled once via a zero-weights matmul
                # so every st-mm can accumulate (start=False) regardless
                # of its column range (group 1 sees kk descending).
                for g in range(st_groups):
                    nc.tensor.matmul(
                        st_banks[g][0:32, :],
                        lhsT=zlhs_sb[:], rhs=xtp_sb[:, 0, :],
                        start=True, stop=False,
                        skip_group_check=True,
                    )
                pend = []          # deferred vector+st work, one pair lag
                count_in = [0] * st_groups
                CHUNKS_G = [0] * st_groups
                for kk in range(KSH):
                    CHUNKS_G[kk * st_groups // KSH] += (
                        1 if N_CTX - 8 * kk <= 256 else 2)

                def emit_zchunk(wk, half, kk, c):
                    base_blk = half * 10
                    n0 = 8 * kk + 256 * c
                    cw = min(256, N_CTX - n0)
                    zc = ppZ.tile([128, 4, 256], f32, tag="z", name="zc")
                    for et in range(4):
                        for dt in range(et + 1):
                            blk = (base_blk + BLK[(et, dt)]) * 128
                            nc.tensor.matmul(
                                zc[:, et, 0:cw],
                                lhsT=wk[:, blk:blk + 128],
                                rhs=xtp_sb[:, dt, n0:n0 + cw],
                                start=(dt == 0),
                                stop=(dt == et),
                                skip_group_check=True,
                            )
                    return zc, n0, cw

                chunk_i = [0]

                def emit_vec_st(kk, zc, n0, cw, last_chunk):
                    g = st_group(kk)
                    route = route_of(chunk_i[0])
                    chunk_i[0] += 1
                    xs = xtp_sb[:, 0:4, n0:n0 + cw]
                    p = spool.tile([128, 4, 256], f16, tag="p16",
                                   name="p16")
                    if route == "direct":
                        nc.vector.scalar_tensor_tensor(
                            out=p[:, :, 0:cw], in0=zc[:, :, 0:cw],
                            scalar=1.0, in1=xs,
                            op0=Alu.mult, op1=Alu.mult)
                    else:
                        c16 = spool2.tile([128, 4, 256], f16, tag="c16",
                                          name="c16")
                        nc.scalar.copy(c16[:, :, 0:cw], zc[:, :, 0:cw])
                        if route == "actpool":
                            nc.gpsimd.tensor_tensor(
                                out=p[:, :, 0:cw], in0=c16[:, :, 0:cw],
                                in1=xs, op=Alu.mult)
                        else:
                            nc.vector.tensor_tensor(
                                out=p[:, :, 0:cw], in0=c16[:, :, 0:cw],
                                in1=xs, op=Alu.mult)
                    sum_eng = (nc.gpsimd if (chunk_i[0] * 11) % 16 < sum_pool
                               else nc.vector)
                    e2 = spool.tile([128, 2, 256], f16, tag="e2", name="e2")
                    sum_eng.tensor_tensor(
                        out=e2[:, :, 0:cw], in0=p[:, 0:2, 0:cw],
                        in1=p[:, 2:4, 0:cw], op=Alu.add)
                    pacc = spool.tile([128, 256], f16, tag="pacc",
                                      name="pacc")
                    sum_eng.tensor_tensor(
                        out=pacc[:, 0:cw], in0=e2[:, 0, 0:cw],
                        in1=e2[:, 1, 0:cw], op=Alu.add)
                    count_in[g] += 1
                    nc.tensor.matmul(
                        st_banks[g][0:32, n0:n0 + cw],
                        lhsT=ones_sb[:, kk % 32, :],
                        rhs=pacc[:, 0:cw],
                        start=False, stop=(count_in[g] == CHUNKS_G[g]),
                        skip_group_check=True,
                    )

                def st_group(kk):
                    return kk * st_groups // KSH

                def flush_pend():
                    while pend:
                        fn = pend.pop(0)
                        fn()

                for j in range(KSH // 2):
                    wk = load_wk_pair(j, split=(2 if j < 2 else 1))
                    if j == 0:
                        load_consts()
                    if j == 10:
                        load_tail_consts()
                    this_pair = []
                    for half, kk in enumerate((j, KSH - 1 - j)):
                        nk = N_CTX - 8 * kk
                        for c in range(1 if nk <= 256 else 2):
                            zc, n0, cw = emit_zchunk(wk, half, kk, c)
                            this_pair.append((kk, zc, n0, cw,
                                              c == (0 if nk <= 256 else 1)))
                    # deferred vector + st work from the previous pair
                    flush_pend()

                    def mk(args):
                        def fn():
                            emit_vec_st(*args)
                        return fn
                    for args in this_pair:
                        pend.append(mk(args))
                flush_pend()

                # ---- scores^T -> scores (PE transposes), gather ----------
                stT = mpool.tile([64, N_CTX], f32, tag="stT", name="stT")
                nc.scalar.copy(stT[0:32, :], st_banks[0][0:32, :])
                nc.scalar.copy(stT[32:64, :], st_banks[1][0:32, :])
                # transpose [64kk, 512i] -> 4x [128i, 64kk]
                tp = ppZ.tile([128, 2, N_CTX], f32, tag="z", name="tp")
                for nt in range(4):
                    nc.tensor.matmul(
                        tp[:, nt % 2, nt // 2 * 64:nt // 2 * 64 + 64],
                        lhsT=stT[:, nt * 128:(nt + 1) * 128],
                        rhs=id32_sb[:],
                        is_transpose=True,
                        skip_group_check=True,
                    )
                for nt in range(4):
                    nc.scalar.copy(
                        scores_sb[:, nt * KSH:(nt + 1) * KSH],
                        tp[:, nt % 2, nt // 2 * 64:nt // 2 * 64 + 64])
                    nc.gpsimd.dma_start(agin_v[:, nt, :],
                                        scores_v[:, nt, :])

                xpwo_sb = tail_c["xpwo"]
                mask_sb = tail_c["mask"]
                if use_collective:
                    nc.gpsimd.collective_compute(
                        "AllToAll",
                        mybir.AluOpType.bypass,
                        replica_groups=[list(range(NCORES))],
                        ins=[agin[:].opt()],
                        outs=[agout[:].opt()],
                    )
                    coll_out = agout
                else:
                    coll_out = agin
                sfull = mpool.tile([RSH, N_CTX], f32, tag="sfull",
                                   name="sfull")
                nc.scalar.dma_start(
                    sfull[:].rearrange("i (r k) -> i r k", r=NCORES),
                    coll_out[:].rearrange("(r i) k -> i r k", r=NCORES),
                )

                # ---- masked softmax + attn tail (as _build_v2) -----------
                nsm = mpool.tile([RSH, N_CTX], f32, tag="sm", name="sm")
                negm = mpool.tile([RSH, 1], f32, tag="negm", name="negm")
                esb = mpool.tile([RSH, N_CTX], f16, tag="esb", name="esb")
                den = mpool.tile([RSH, 1], f32, tag="den", name="den")
                nc.vector.tensor_tensor(
                    out=nsm[:], in0=sfull[:], in1=mask_sb[:],
                    op=Alu.subtract)
                nc.vector.reduce_max(
                    negm[:], nsm[:], axis=mybir.AxisListType.X,
                    negate=True)
                nc.scalar.activation(
                    esb[:], nsm[:], Act.Exp, bias=negm[:], scale=1.0,
                    accum_out=den[:])
                rden = mpool.tile([RSH, 1], f32, tag="rden", name="rden")
                nc.vector.reciprocal(rden[:], den[:])

                pt = ppS.tile([128, 4, RSH], f16, tag="st", name="ptT")
                at_sb = []
                for kt in range(4):
                    nc.tensor.matmul(
                        pt[:, kt, :],
                        lhsT=esb[:, kt * 128:(kt + 1) * 128],
                        rhs=ident_sb[:],
                        is_transpose=True,
                        skip_group_check=True,
                    )
                for kt in range(4):
                    at = mpool.tile([128, RSH], f16, tag=f"at{kt}",
                                    name=f"at{kt}")
                    nc.scalar.copy(at[:], pt[:, kt, :])
                    at_sb.append(at)

                ot_sb = []
                for et in range(4):
                    op = ppZ.tile([128, 2, N_CTX], f32, tag="z", name="op")
                    for kt in range(4):
                        nc.tensor.matmul(
                            op[:, 0, 0:RSH],
                            lhsT=xpwo_sb[:, kt, et * 128:(et + 1) * 128],
                            rhs=at_sb[kt][:],
                            start=(kt == 0),
                            stop=(kt == 3),
                        )
                    ot = mpool.tile([128, RSH], f16, tag=f"ot{et}",
                                    name=f"ot{et}")
                    nc.scalar.copy(ot[:], op[:, 0, 0:RSH])
                    ot_sb.append(ot)

                ypz = ppZ.tile([128, 2, N_CTX], f32, tag="z", name="ypz")
                for et in range(4):
                    nc.tensor.matmul(
                        ypz[0:RSH, 0, :],
                        lhsT=ot_sb[et][:],
                        rhs=xpwo_sb[:, 4 + et, :],
                        start=(et == 0),
                        stop=(et == 3),
                    )
                y_sb = mpool.tile([RSH, D], f32, tag="y_sb", name="y_sb")
                nc.scalar.mul(y_sb[:], ypz[0:RSH, 0, :], rden[:])
                nc.scalar.dma_start(out_t[:], y_sb[:])

            def alloc_st():
                return [ppS.tile([32, N_CTX], f32, tag="st",
                                 name=f"st{g}") for g in range(st_groups)]

            if timing_loop == -1:
                tmp = nc.alloc_registers("niter_reg", mybir.ALL_ENGINES)
                nc.regs_load(tmp, niter_t[0:1, 0:1])
                nval = nc.snap(tmp, donate=True, min_val=0, max_val=1024)
                with tc.For_i(0, nval, 1):
                    body(alloc_st())
            elif timing_loop:
                with tc.For_i(0, timing_loop, 1):
                    body(alloc_st())
            else:
                body(alloc_st())

    nc.compile()
    return nc


def _pack_blocks(Wm):
    """[KSH, 512, 512] fp32 -> [KSH, 128, PACKW] fp16 block pack for the
    W-stationary stage A: per k, the 10 upper-block-triangle [128,128]
    blocks of U' in (et, dt) order (et asc, dt asc within et)."""
    U = np.triu(Wm + Wm.transpose(0, 2, 1), 1)
    idx = np.arange(D)
    U[:, idx, idx] = Wm[:, idx, idx]
    pack = np.empty((Wm.shape[0], 128, PACKW), np.float16)
    i = 0
    for et in range(4):
        for dt in range(et + 1):
            pack[:, :, i * 128:(i + 1) * 128] = \
                U[:, 128 * dt:128 * dt + 128, 128 * et:128 * et + 128]
            i += 1
    return pack


def _pack_upper(Wm):
    """[KSH, 512, 512] fp32 -> [KSH, 128, PACKW] fp16 upper-tri pack.

    U' = triu(W + W^T, 1) + diag(W); block dt holds U'[128dt+p, 128dt:512].
    """
    U = np.triu(Wm + Wm.transpose(0, 2, 1), 1)
    idx = np.arange(D)
    U[:, idx, idx] = Wm[:, idx, idx]
    pack = np.empty((KSH, 128, PACKW), np.float16)
    for dt in range(4):
        lo = 128 * dt
        pack[:, :, OFFS[dt]:OFFS[dt] + SPANS[dt]] = U[:, lo:lo + 128, lo:D]
    return pack


def _make_in_maps(x, W_bi, W_out, stage_a="causal"):
    x = np.ascontiguousarray(np.asarray(x, dtype=np.float32))
    W_bi = np.asarray(W_bi, dtype=np.float32)
    W_out = np.asarray(W_out, dtype=np.float32)
    # x row-major packed [p, nt, d] = x[128*nt + p, d]
    xpk = np.ascontiguousarray(x.reshape(4, 128, D).transpose(1, 0, 2))
    x16k = xpk.astype(np.float16)
    # xtp[p, dt, n] = x[n, 128*dt + p]
    xtp16 = np.ascontiguousarray(
        x.T.reshape(4, 128, N_CTX).transpose(1, 0, 2)).astype(np.float16)
    # interleaved k-sharding: core m owns global columns k = 8*kk + m.
    # After the AllToAll gather, score column position p = r*64 + kk
    # holds global k = 8*kk + r, so X rows and the causal mask are
    # permuted to match.
    perm = np.array([8 * (p % KSH) + p // KSH for p in range(N_CTX)])
    xperm = x[perm]
    woutt = W_out.T
    # xpwo[p, 0:4, :] = xperm blocks, xpwo[p, 4+et, :] = W_out^T blocks
    xpwo = np.empty((128, 8, D), np.float16)
    xpwo[:, 0:4, :] = xperm.reshape(4, 128, D).transpose(1, 0, 2)
    xpwo[:, 4:8, :] = woutt.reshape(4, 128, D).transpose(1, 0, 2)
    kcol = perm[None, :]                       # global k at position p
    in_maps = []
    for m in range(NCORES):
        pack = _pack_upper(np.ascontiguousarray(W_bi[m::NCORES]))
        # pair layout: [j] = concat(pack[j], pack[63-j]) along the free dim
        pairs = np.concatenate([pack[:KSH // 2], pack[:KSH // 2 - 1:-1]],
                               axis=2)
        rows = np.arange(m * RSH, (m + 1) * RSH)[:, None]
        # negated mask: 0 where allowed, +1e30 where masked
        mask = np.where(kcol <= rows, 0.0, -NEG_INF).astype(np.float32)
        pack3 = _pack_blocks(np.ascontiguousarray(W_bi[m::NCORES]))
        pairs3 = np.concatenate([pack3[:KSH // 2], pack3[:KSH // 2 - 1:-1]],
                                axis=2)
        ones32 = np.zeros((128, 32, 32), np.float16)
        for c in range(32):
            ones32[:, c, c] = 1.0
        in_maps.append({
            "x": xpk,
            "x16": x16k,
            "xpwo": xpwo,
            "xtp": xtp16,
            "wbi": np.ascontiguousarray(pairs),
            "wbi3": np.ascontiguousarray(pairs3),
            "mask": np.ascontiguousarray(mask),
            "ident": np.eye(64, dtype=np.float16),
            "ident32": np.eye(64, dtype=np.float32),
            "ones32": ones32,
        })
    return in_maps


def kernel(x, W_bi, W_out):
    global _nc_cache
    import time as _time
    from concourse.bass_utils import run_bass_kernel_spmd

    if _nc_cache is None:
        _nc_cache = _build_v2(**V2_KW)
    nc = _nc_cache
    in_maps = _make_in_maps(x, W_bi, W_out, stage_a=STAGE_A)
    last_exc = None
    for attempt in range(3):
        try:
            res = run_bass_kernel_spmd(nc, in_maps, core_ids=list(range(NCORES)),
                                       trace=False)
            break
        except Exception as e:  # transient NRT/axon wedges recover on retry
            last_exc = e
            _time.sleep(5.0 * (attempt + 1))
    else:
        raise last_exc
    out = np.concatenate([res.results[m]["out"] for m in range(NCORES)], axis=0)
    return np.ascontiguousarray(out, dtype=np.float32)



# revision 27
# speedup vs baseline: 1.0046x; 1.0046x over previous
"""Bilinear causal attention (nn_Attention_34772055228779) on 8 trn2 cores.

reference:
  scores[i,k] = x[i] @ W_bi[k] @ x[i]          [512, 512]
  attn = softmax(scores + causal_mask, axis=1)
  out  = (attn @ x) @ W_out.T                  [512, 512]

Device strategy (tensor-parallel over score columns, per sharding hint):
  core m holds the k-interleaved shard W_bi[m::8] (64 local columns).

  Only the symmetric part of W_bi[k] contributes to x^T W x, so the host
  packs U'_k = triu(W_k + W_k^T, 1) + diag(W_k)  (exact identity:
  x^T U' x = x^T W x).  U' is upper-triangular, so the d-row-block dt only
  has nonzeros in columns e >= 128*dt: the four matmul rhs spans are
  512/384/256/128 instead of 4x512 (37.5%% less PE work), and the packed
  fp16 stream is 320 KiB/k = 20 MiB/core instead of 64 MiB fp32.

  stage A (_build_v2): for each local k: Y_k = X16 @ U'16_k (fp16
           matmuls, fp32 PSUM, lhsT = X^T resident, 7 PSUM banks),
           scores[:, k] = rowsum(Y_k * X) -- mostly one fused DVE
           scalar_tensor_tensor per row-tile (f16 throwaway out: the
           accum_out sums internal fp32 products, and halving the dead
           SBUF write bandwidth is worth ~2 us on HW); 7 of every 40
           tiles are routed [ACT copy f16 -> Pool mult -> ACT accum] to
           keep DVE just under the PE.  Causally dead row-tiles are skipped
           (SPMD-uniform bound with the k-interleaved sharding).
  Startup: first weight-pair DMAs split in half; x/ident residents ride
           the Pool SWDGE queue so the SP queue stays clear for the
           weight stream.
  Gather:  score columns are gathered to DRAM in 4 chunks as their
           stts complete; the final chunk is only 4 columns, so the
           AllToAll launches almost immediately after the last stt.
  tail:    masked softmax rows (DVE sub/max + ACT exp with fused denom
           accum; Exp table pre-warmed at body start), A^T via fp16 PE
           transposes into PSUM (saves ~8 us vs xbar DMA transposes),
           O^T = X^T A^T, Y = O @ W_out^T, per-partition 1/den folded
           into the final copy, DMA 64 rows out.
  host:    concatenates the 8 row blocks.

Measured (this hardware, single-NEFF dynamic-trip-count slope, stubbed
collective): 130-132 us/iter vs 147-150 us/iter for the previous
kernel; timeline-sim (cost-model) totals 113.6 vs 143.4 us.

Tried and rejected (measured, do not repeat blindly):
  - W-stationary stage A w/ exact causal streaming (_build_v3): PE busy
    drops 91->84 us but the e-block-sum vector work + longer per-chunk
    dependency chains add ~25 us of pipeline stalls (sim 136 vs 113.6).
  - tail consts / xtp on the ACT hwdge queue: ACT SEQ spends 667 ns
    issuing each DMA and delays the routed-path copies (HW: +4 us).
  - weight stream alternating SP/Pool queues: Pool SWDGE descriptor gen
    runs on the Pool engine (sim: +2..13 us).
  - routing more than 7/40 stts off DVE: the 3-op routed chain holds
    yp PSUM banks longer; bank pressure beats engine balance.
  - fp8 weights (score noise ~0.8 logits vs ~0.01 budget), lhsT-sharing
    matmul order (HW: no effect), 2D sharding (doubles weight DMA).
"""
import numpy as np

N_CTX = 512
D = 512
NCORES = 8
KSH = N_CTX // NCORES      # 64 score columns per core
RSH = N_CTX // NCORES      # 64 output rows per core
NEG_INF = -1e30
STAGE_A = "causal"   # "causal" skips fully-masked row-tiles (k-interleaved)

# upper-triangular pack: per dt row-block, columns [128*dt, 512)
SPANS = [512, 384, 256, 128]
OFFS = [0, 512, 896, 1152]          # column offset of block dt in the pack
PACKW = 1280                         # total packed width per partition

_nc_cache = None

# best Phase-1 configuration (sim-tuned)
V2_KW = dict(route_n=3, route_grp=40, route_slots=(3, 14, 25, 31, 9, 20, 36),
             xq_act="pool", gather4=True, sfull_act=True, warm_exp=True,
             wbufs=6, ppa_bufs=7, scr16=True)


def _build(timing_loop=0, use_collective=True, num_devices=NCORES,
           stage_a="causal", wbufs=4, stt_split=True, softmax_fused=False,
           gather_3d=True, debug_scores=False, route_red="act"):
    # NOTE: softmax_fused=True (tensor_tensor_reduce min) compiles but
    # crashes the exec unit on real TRN2 hardware -- keep it off.
    """Build the Bass module.

    timing_loop=R>0 wraps the whole per-core body in a hardware For_i loop
    (R iterations) for slope timing; collectives can't sit in control flow,
    so timing variants pass use_collective=False (the gather DMA then reads
    the pre-collective buffer -- wrong data, identical shapes/costs).
    """
    import concourse.mybir as mybir
    import concourse.tile as tile
    from concourse import bacc

    f32 = mybir.dt.float32
    f16 = mybir.dt.float16
    Alu = mybir.AluOpType
    Act = mybir.ActivationFunctionType

    nc = bacc.Bacc(
        "TRN2", target_bir_lowering=False, debug=False,
        enable_asserts=False, num_devices=num_devices,
    )

    # x row-major packed [p, nt, d] in f32 and f16: one DMA each
    x_t = nc.dram_tensor("x", [128, 4, D], f32, kind="ExternalInput").ap()
    x16_t = nc.dram_tensor("x16", [128, 4, D], f16, kind="ExternalInput").ap()
    # tail constants packed [p, 8, e] f16: [:,0:4] = column-permuted X rows
    # (k-interleaved layout) for attn @ X, [:,4:8] = W_out^T blocks
    xpwo_t = nc.dram_tensor("xpwo", [128, 8, D], f16,
                            kind="ExternalInput").ap()
    # X^T packed [p, dt, n]: one DMA loads all four lhsT d-blocks
    xtp_t = nc.dram_tensor("xtp", [128, 4, N_CTX], f16,
                           kind="ExternalInput").ap()
    # W pairs: [j] holds packed U' for columns kk=j and kk=63-j
    wbi_t = nc.dram_tensor("wbi", [KSH // 2, 128, 2 * PACKW], f16,
                           kind="ExternalInput").ap()
    # negated additive mask: 0 where allowed, +1e30 where causally masked
    mask_t = nc.dram_tensor("mask", [RSH, N_CTX], f32, kind="ExternalInput").ap()
    niter_t = (nc.dram_tensor("niter", [1, 1], mybir.dt.int32,
                              kind="ExternalInput").ap()
               if timing_loop == -1 else None)
    out_t = nc.dram_tensor("out", [RSH, D], f32, kind="ExternalOutput").ap()
    dbg_t = (nc.dram_tensor("dbg", [128, 4 * KSH], f32,
                            kind="ExternalOutput").ap()
             if debug_scores else None)

    with tile.TileContext(nc) as tc:
        with (
            tc.tile_pool(name="const", bufs=1) as cpool,
            tc.tile_pool(name="tailc", bufs=2) as tcpool,
            tc.tile_pool(name="wstream", bufs=wbufs) as wpool,
            tc.tile_pool(name="scratch", bufs=3) as spool,
            tc.tile_pool(name="scratch2", bufs=3) as spool2,
            tc.tile_pool(name="small", bufs=1) as mpool,
            tc.tile_pool(name="psA", bufs=6, space="PSUM") as ppA,
            tc.tile_pool(name="psB", bufs=2, space="PSUM") as ppB,
            tc.tile_pool(name="dram", bufs=1, space="DRAM") as dpool,
        ):
            # ---- resident loads (outside any timing loop) -----------------
            # xt first (single packed DMA): the first matmul only needs
            # xt + wk0, so the x/x16 loads (needed ~2.6us later by the
            # first stt) are issued after the first wk DMA to cut the
            # startup serial chain.
            xtp_sb = cpool.tile([128, 4, N_CTX], f16, tag="xtp", name="xtp")
            nc.sync.dma_start(xtp_sb[:], xtp_t[:])
            xpk_sb = cpool.tile([128, 4, N_CTX], f32, tag="xpk", name="xpk")
            x16k_sb = cpool.tile([128, 4, N_CTX], f16, tag="x16k",
                                 name="x16k")

            def load_x_resident():
                nc.sync.dma_start(xpk_sb[:], x_t[:])
                nc.sync.dma_start(x16k_sb[:], x16_t[:])
            # single score accumulator tile, column nt*KSH + kk
            scores_sb = cpool.tile([128, 4 * KSH], f32, tag="sc", name="sc")
            # skipped (nt, kk) cells are never written; zero them so no
            # NaN bit-patterns survive into exp() past the additive mask
            nc.gpsimd.memset(scores_sb[:], 0.0)
            agin = dpool.tile([N_CTX, KSH], f32, tag="agin")
            agout = dpool.tile([N_CTX, KSH], f32, tag="agout")
            agin_v = agin[:].rearrange("(t p) k -> p t k", p=128)
            scores_v = scores_sb[:].rearrange("p (t k) -> p t k", t=4)

            def load_wk_pair(j):
                # one DMA covers both columns of the pair (j, 63-j)
                wk = wpool.tile([128, 2 * PACKW], f16, tag="wk", name="wk")
                nc.sync.dma_start(wk[:], wbi_t[j])
                return wk

            # stt engine split: only DVE can reduce straight from PSUM
            # (Pool has no PSUM access and TensorScalarPtr is not a legal
            # Pool opcode).  A share of tiles is therefore routed
            #   ACT:  yp (PSUM f32) -> y16 (SBUF f16)
            #   Pool: prod16 = y16 * x16          (TensorTensor, SBUF)
            #   ACT:  Copy(prod16) with accum_out -> scores column
            # Costs: DVE stt ~658 ns; ACT ~2x660 ns and Pool ~840 ns per
            # routed tile.  6 of every 20 tiles (spread, not consecutive,
            # so DVE never sits idle for long) puts DVE ~76us, ACT ~64us
            # and Pool ~40us, all under the ~90us PE stage-A floor.
            POOL_SLOTS = {3, 6, 9, 13, 16, 19}
            stt_state = {"i": 0}

            def emit_stt(yp, nt, kk):
                if stt_split:
                    use_dve = (stt_state["i"] % 20) not in POOL_SLOTS
                    stt_state["i"] += 1
                else:
                    use_dve = True
                col = nt * KSH + kk
                if use_dve:
                    scr = spool.tile([128, D], f32, tag="stt_out", name="scr")
                    nc.vector.scalar_tensor_tensor(
                        out=scr[:], in0=yp[:], scalar=1.0,
                        in1=xpk_sb[:, nt, :],
                        op0=Alu.mult, op1=Alu.mult,
                        accum_out=scores_sb[:, col:col + 1],
                    )
                else:
                    y16 = spool2.tile([128, D], f16, tag="y16", name="y16")
                    nc.scalar.copy(y16[:], yp[:])
                    prod = spool2.tile([128, D], f16, tag="prod", name="prod")
                    nc.gpsimd.tensor_tensor(
                        out=prod[:], in0=y16[:], in1=x16k_sb[:, nt, :],
                        op=Alu.mult)
                    if route_red == "dve":
                        nc.vector.tensor_reduce(
                            scores_sb[:, col:col + 1], prod[:],
                            axis=mybir.AxisListType.X, op=Alu.add)
                    else:
                        scr = spool2.tile([128, D], f16, tag="scr16",
                                          name="scr16")
                        nc.scalar.activation(
                            scr[:], prod[:], Act.Copy, bias=0.0, scale=1.0,
                            accum_out=scores_sb[:, col:col + 1])

            def stage_a_tri(load_tail_consts):
                # causal: with k-interleaved sharding (global k = 8*kk + m),
                # row-tiles nt < kk//16 are fully masked for column kk on
                # EVERY core, so the skip bound is SPMD-uniform.
                #
                # Column order pairs kk with 63-kk: every pair is exactly 5
                # kept row-tiles of PE work against 2 wk DMAs, so the DMA
                # stream never outpaces nor starves the PE (a plain
                # ascending order leaves PE idle behind DMA for the late,
                # 1-tile columns).
                for j in range(KSH // 2):
                    wk = load_wk_pair(j)
                    if j == 0:
                        # must precede the first stt in program order: the
                        # dependency tracker only orders reads after writes
                        # that were already emitted
                        load_x_resident()
                    if j == 10:
                        # late enough that the wk pair stream has built a
                        # surplus on the shared DMA engines; the constants
                        # still land ~70us before the tail reads them
                        load_tail_consts()
                    for half, kk in enumerate((j, KSH - 1 - j)):
                        base = half * PACKW
                        nt_lo = (kk // 16) if stage_a == "causal" else 0
                        for nt in range(nt_lo, 4):
                            yp = ppA.tile([128, D], f32, tag="yp", name="yp")
                            for dt in range(4):
                                span = SPANS[dt]
                                nc.tensor.matmul(
                                    yp[:, D - span:D],
                                    lhsT=xtp_sb[:, dt,
                                                nt * 128:(nt + 1) * 128],
                                    rhs=wk[:, base + OFFS[dt]:
                                           base + OFFS[dt] + span],
                                    start=(dt == 0),
                                    stop=(dt == 3),
                                    skip_group_check=True,
                                )
                            emit_stt(yp, nt, kk)
                    if j == 15 and gather_3d:
                        # columns {0..15, 48..63} are final: start their
                        # DRAM gather under the remaining compute.  On the
                        # Pool SWDGE queue so the wait on those columns'
                        # stts never blocks the SP weight-stream queue.
                        nc.gpsimd.dma_start(
                            agin_v[:, :, 0:16], scores_v[:, :, 0:16])
                        nc.gpsimd.dma_start(
                            agin_v[:, :, 48:64], scores_v[:, :, 48:64])

            def body():
                # tail constants, double-buffered (bufs=2) so the timing
                # loop's next iteration can re-load them without a
                # write-after-read stall against this iteration's tail
                tail_c = {}

                def load_tail_consts():
                    tail_c["xpwo"] = tcpool.tile(
                        [128, 8, N_CTX], f16, tag="xpwo", name="xpwo")
                    tail_c["mask"] = tcpool.tile(
                        [RSH, N_CTX], f32, tag="mask", name="mask")
                    nc.sync.dma_start(tail_c["xpwo"][:], xpwo_t[:])
                    nc.sync.dma_start(tail_c["mask"][:], mask_t[:])

                # ---- stage A: local score columns -------------------------
                stage_a_tri(load_tail_consts)
                xpwo_sb = tail_c["xpwo"]
                mask_sb = tail_c["mask"]

                # ---- AllToAll: shard columns -> shard rows ----------------
                # (columns {0..15, 48..63} were already gathered mid-stage-A)
                # Gather/scatter DMAs ride the Pool SWDGE queue, same as the
                # collective, keeping the SP queue free for the next
                # iteration's weight stream.
                if gather_3d:
                    nc.gpsimd.dma_start(
                        agin_v[:, :, 16:48], scores_v[:, :, 16:48])
                else:
                    for nt in range(4):
                        nc.gpsimd.dma_start(
                            agin[nt * 128:(nt + 1) * 128, :],
                            scores_sb[:, nt * KSH:(nt + 1) * KSH])
                if use_collective:
                    nc.gpsimd.collective_compute(
                        "AllToAll",
                        mybir.AluOpType.bypass,
                        replica_groups=[list(range(NCORES))],
                        ins=[agin[:].opt()],
                        outs=[agout[:].opt()],
                    )
                    coll_out = agout
                else:
                    coll_out = agin
                # rows of the full score matrix for this core: [64, 512]
                sfull = mpool.tile([RSH, N_CTX], f32, tag="sfull", name="sfull")
                nc.gpsimd.dma_start(
                    sfull[:].rearrange("i (r k) -> i r k", r=NCORES),
                    coll_out[:].rearrange("(r i) k -> i r k", r=NCORES),
                )

                # ---- masked softmax over the 64 rows ----------------------
                # fused mask+max: nsm = negmask - scores (so masked cells are
                # ~+1e30 and min(nsm) = -max of the allowed scores), then
                # exp(-nsm + bias) on ACT.  The 1/denominator is folded into
                # the final output copy as a per-partition ACT scale, keeping
                # the reciprocal off the critical path.
                nsm = mpool.tile([RSH, N_CTX], f32, tag="sm", name="sm")
                negm = mpool.tile([RSH, 1], f32, tag="negm", name="negm")
                esb = mpool.tile([RSH, N_CTX], f16, tag="esb", name="esb")
                den = mpool.tile([RSH, 1], f32, tag="den", name="den")
                if softmax_fused:
                    # nsm = negmask - s (masked cells ~ +1e30), negm =
                    # min(nsm) = -max over allowed, exp(-nsm + negm)
                    nc.vector.tensor_tensor_reduce(
                        out=nsm[:], in0=mask_sb[:], in1=sfull[:], scale=1.0,
                        scalar=float(-NEG_INF), op0=Alu.subtract, op1=Alu.min,
                        accum_out=negm[:])
                    nc.scalar.activation(
                        esb[:], nsm[:], Act.Exp, bias=negm[:], scale=-1.0,
                        accum_out=den[:])
                else:
                    # sm = s - negmask (masked cells ~ -1e30)
                    nc.vector.tensor_tensor(
                        out=nsm[:], in0=sfull[:], in1=mask_sb[:],
                        op=Alu.subtract)
                    nc.vector.reduce_max(
                        negm[:], nsm[:], axis=mybir.AxisListType.X,
                        negate=True)
                    nc.scalar.activation(
                        esb[:], nsm[:], Act.Exp, bias=negm[:], scale=1.0,
                        accum_out=den[:])
                rden = mpool.tile([RSH, 1], f32, tag="rden", name="rden")
                nc.vector.reciprocal(rden[:], den[:])

                # ---- A^T via xbar DMA transpose: [64, 512] -> 4x [128, 64]
                # (unnormalized fp16 exp weights; dispatched on the ACT
                # HWDGE queue so same-engine ordering after the exp makes
                # the chain wait-free)
                at_sb = []
                for kt in range(4):
                    at = mpool.tile([128, RSH], f16, tag=f"at{kt}",
                                    name=f"at{kt}")
                    nc.scalar.dma_start_transpose(
                        at[:], esb[:, kt * 128:(kt + 1) * 128])
                    at_sb.append(at)

                # ---- O^T = X^T @ A^T : [512(e), 64(i)] --------------------
                ot_sb = []
                for et in range(4):
                    op = ppB.tile([128, 512], f32, tag="tail", name="op")
                    for kt in range(4):
                        nc.tensor.matmul(
                            op[:, 0:RSH],
                            lhsT=xpwo_sb[:, kt, et * 128:(et + 1) * 128],
                            rhs=at_sb[kt][:],
                            start=(kt == 0),
                            stop=(kt == 3),
                        )
                    ot = mpool.tile([128, RSH], f16, tag=f"ot{et}",
                                    name=f"ot{et}")
                    nc.scalar.copy(ot[:], op[:, 0:RSH])
                    ot_sb.append(ot)

                # ---- Y = O @ W_out^T : [64(i), 512(f)] --------------------
                ypz = ppB.tile([128, 512], f32, tag="tail", name="ypz")
                for et in range(4):
                    nc.tensor.matmul(
                        ypz[0:RSH, :],
                        lhsT=ot_sb[et][:],
                        rhs=xpwo_sb[:, 4 + et, :],
                        start=(et == 0),
                        stop=(et == 3),
                    )
                # final copy normalizes the softmax: per-partition 1/den
                y_sb = mpool.tile([RSH, D], f32, tag="y_sb", name="y_sb")
                nc.scalar.mul(y_sb[:], ypz[0:RSH, :], rden[:])
                nc.scalar.dma_start(out_t[:], y_sb[:])
                if debug_scores:
                    nc.sync.dma_start(dbg_t[:], scores_sb[:])

            if timing_loop == -1:
                # dynamic trip count from the niter input: one NEFF serves
                # every loop length, so slope measurements compare runs of
                # the SAME executable (per-executable launch offsets cancel)
                tmp = nc.alloc_registers("niter_reg", mybir.ALL_ENGINES)
                nc.regs_load(tmp, niter_t[0:1, 0:1])
                nval = nc.snap(tmp, donate=True, min_val=0, max_val=1024)
                with tc.For_i(0, nval, 1):
                    body()
            elif timing_loop:
                with tc.For_i(0, timing_loop, 1):
                    body()
            else:
                body()

    nc.compile()
    return nc


def _build_v2(timing_loop=0, use_collective=True, num_devices=NCORES,
              wbufs=6, route_n=6, route_grp=20, pe_transpose=True,
              gather3=True, split_first=2, ppa_bufs=7, route_red="act",
              xq_act=True, gather4=True, sfull_act=True, warm_exp=True,
              route_slots=None, tailc_act=False, xtp_act=False,
              wk_alt=0, xtp_interleave=False, last_gather_act=False,
              tail_route=(), x16_stt=False, scr16=False):
    """Phase-1 rework of _build: 7-bank stage-A PSUM (tail reuses them),
    retuned stt routing (ACT copy -> Pool tt -> ACT accum), split first
    weight DMAs (startup latency), 3-chunk score gather, PE-transpose tail.
    """
    import concourse.mybir as mybir
    import concourse.tile as tile
    from concourse import bacc

    f32 = mybir.dt.float32
    f16 = mybir.dt.float16
    Alu = mybir.AluOpType
    Act = mybir.ActivationFunctionType

    nc = bacc.Bacc(
        "TRN2", target_bir_lowering=False, debug=False,
        enable_asserts=False, num_devices=num_devices,
    )

    x_t = nc.dram_tensor("x", [128, 4, D], f32, kind="ExternalInput").ap()
    x16_t = nc.dram_tensor("x16", [128, 4, D], f16, kind="ExternalInput").ap()
    xpwo_t = nc.dram_tensor("xpwo", [128, 8, D], f16,
                            kind="ExternalInput").ap()
    xtp_t = nc.dram_tensor("xtp", [128, 4, N_CTX], f16,
                           kind="ExternalInput").ap()
    wbi_t = nc.dram_tensor("wbi", [KSH // 2, 128, 2 * PACKW], f16,
                           kind="ExternalInput").ap()
    mask_t = nc.dram_tensor("mask", [RSH, N_CTX], f32,
                            kind="ExternalInput").ap()
    ident_t = nc.dram_tensor("ident", [64, 64], f16,
                             kind="ExternalInput").ap()
    niter_t = (nc.dram_tensor("niter", [1, 1], mybir.dt.int32,
                              kind="ExternalInput").ap()
               if timing_loop == -1 else None)
    out_t = nc.dram_tensor("out", [RSH, D], f32, kind="ExternalOutput").ap()

    with tile.TileContext(nc) as tc:
        with (
            tc.tile_pool(name="const", bufs=1) as cpool,
            tc.tile_pool(name="tailc", bufs=2) as tcpool,
            tc.tile_pool(name="wstream", bufs=wbufs) as wpool,
            tc.tile_pool(name="scratch", bufs=3) as spool,
            tc.tile_pool(name="scratch2", bufs=3) as spool2,
            tc.tile_pool(name="small", bufs=1) as mpool,
            tc.tile_pool(name="psA", bufs=ppa_bufs, space="PSUM") as ppA,
            tc.tile_pool(name="psB", bufs=1, space="PSUM") as ppB,
            tc.tile_pool(name="dram", bufs=1, space="DRAM") as dpool,
        ):
            # ---- resident loads ------------------------------------------
            # xtp rides the ACT queue in two halves so the SP queue opens
            # with the first weight DMA and the first matmul (needing only
            # xtp[:, 0]) starts ~1.5us earlier
            xtp_sb = cpool.tile([128, 4, N_CTX], f16, tag="xtp", name="xtp")
            if xtp_act:
                nc.scalar.dma_start(xtp_sb[:, 0:2, :], xtp_t[:, 0:2, :])
                nc.scalar.dma_start(xtp_sb[:, 2:4, :], xtp_t[:, 2:4, :])
            elif xtp_interleave:
                # only the dt 0/1 half ahead of the first weight DMA; the
                # dt 2/3 half is issued right after it (stage_a j==0)
                nc.sync.dma_start(xtp_sb[:, 0:2, :], xtp_t[:, 0:2, :])
            else:
                nc.sync.dma_start(xtp_sb[:], xtp_t[:])
            xpk_sb = cpool.tile([128, 4, N_CTX], f32, tag="xpk", name="xpk")
            x16k_sb = cpool.tile([128, 4, N_CTX], f16, tag="x16k",
                                 name="x16k")
            ident_sb = cpool.tile([64, 64], f16, tag="ident", name="ident")

            def load_x_resident():
                # ACT hwdge queue: keeps the SP queue clear for the wk
                # stream (x loads there stalled PE ~5us at startup)
                eng = {"act": nc.scalar, "pool": nc.gpsimd,
                       "sp": nc.sync}[xq_act if isinstance(xq_act, str)
                                      else ("act" if xq_act else "sp")]
                eng.dma_start(xpk_sb[:], x_t[:])
                eng.dma_start(x16k_sb[:], x16_t[:])
                eng.dma_start(ident_sb[:], ident_t[:])
            scores_sb = cpool.tile([128, 4 * KSH], f32, tag="sc", name="sc")
            nc.gpsimd.memset(scores_sb[:], 0.0)
            agin = dpool.tile([N_CTX, KSH], f32, tag="agin")
            agout = dpool.tile([N_CTX, KSH], f32, tag="agout")
            agin_v = agin[:].rearrange("(t p) k -> p t k", p=128)
            scores_v = scores_sb[:].rearrange("p (t k) -> p t k", t=4)

            def load_wk_pair(j, split=1):
                wk = wpool.tile([128, 2 * PACKW], f16, tag="wk", name="wk")
                eng = nc.gpsimd if (wk_alt and j % wk_alt == wk_alt - 1) \
                    else nc.sync
                if split == 1:
                    eng.dma_start(wk[:], wbi_t[j])
                else:
                    eng.dma_start(wk[:, 0:PACKW], wbi_t[j][:, 0:PACKW])
                    eng.dma_start(wk[:, PACKW:], wbi_t[j][:, PACKW:])
                return wk

            # stt: DVE direct, or routed [ACT copy f16 -> Pool tt -> ACT
            # accum].  route_n of every route_grp tiles take the routed path.
            if route_slots is not None:
                ROUTE_SLOTS = set(route_slots)
            else:
                ROUTE_SLOTS = set()
                if route_n:
                    step = route_grp / route_n
                    ROUTE_SLOTS = {int(step * i + step / 2)
                                   for i in range(route_n)}
            stt_state = {"i": 0}

            def emit_stt(yp, nt, kk):
                i = stt_state["i"]
                use_dve = ((i % route_grp) not in ROUTE_SLOTS
                           and i not in tail_route)
                stt_state["i"] += 1
                col = nt * KSH + kk
                if use_dve:
                    scr = spool.tile([128, D],
                                     f16 if scr16 else f32,
                                     tag="stt_out", name="scr")
                    nc.vector.scalar_tensor_tensor(
                        out=scr[:], in0=yp[:], scalar=1.0,
                        in1=(x16k_sb if x16_stt else xpk_sb)[:, nt, :],
                        op0=Alu.mult, op1=Alu.mult,
                        accum_out=scores_sb[:, col:col + 1],
                    )
                else:
                    y16 = spool2.tile([128, D], f16, tag="y16", name="y16")
                    nc.scalar.copy(y16[:], yp[:])
                    prod = spool2.tile([128, D], f16, tag="prod", name="prod")
                    nc.gpsimd.tensor_tensor(
                        out=prod[:], in0=y16[:], in1=x16k_sb[:, nt, :],
                        op=Alu.mult)
                    if route_red == "dve":
                        nc.vector.tensor_reduce(
                            scores_sb[:, col:col + 1], prod[:],
                            axis=mybir.AxisListType.X, op=Alu.add)
                    else:
                        scr = spool2.tile([128, D], f16, tag="scr16",
                                          name="scr16")
                        nc.scalar.activation(
                            scr[:], prod[:], Act.Copy, bias=0.0, scale=1.0,
                            accum_out=scores_sb[:, col:col + 1])

            def stage_a(load_tail_consts):
                for j in range(KSH // 2):
                    wk = load_wk_pair(j, split=(2 if j < split_first else 1))
                    if j == 0:
                        if xtp_interleave and not xtp_act:
                            nc.sync.dma_start(xtp_sb[:, 2:4, :],
                                              xtp_t[:, 2:4, :])
                        load_x_resident()
                    if j == 10:
                        load_tail_consts()
                    for half, kk in enumerate((j, KSH - 1 - j)):
                        base = half * PACKW
                        nt_lo = kk // 16
                        for nt in range(nt_lo, 4):
                            yp = ppA.tile([128, D], f32, tag="yp", name="yp")
                            for dt in range(4):
                                span = SPANS[dt]
                                nc.tensor.matmul(
                                    yp[:, D - span:D],
                                    lhsT=xtp_sb[:, dt,
                                                nt * 128:(nt + 1) * 128],
                                    rhs=wk[:, base + OFFS[dt]:
                                           base + OFFS[dt] + span],
                                    start=(dt == 0),
                                    stop=(dt == 3),
                                    skip_group_check=True,
                                )
                            emit_stt(yp, nt, kk)
                    if gather3 and j == 15:
                        # columns {0..15, 48..63} final: gather now
                        nc.gpsimd.dma_start(
                            agin_v[:, :, 0:16], scores_v[:, :, 0:16])
                        nc.gpsimd.dma_start(
                            agin_v[:, :, 48:64], scores_v[:, :, 48:64])
                    if gather3 and j == 23:
                        # columns {16..23, 40..47} final
                        nc.gpsimd.dma_start(
                            agin_v[:, :, 16:24], scores_v[:, :, 16:24])
                        nc.gpsimd.dma_start(
                            agin_v[:, :, 40:48], scores_v[:, :, 40:48])
                    if gather3 and gather4 and j == 29:
                        # columns {24..29, 34..39} final
                        nc.gpsimd.dma_start(
                            agin_v[:, :, 24:30], scores_v[:, :, 24:30])
                        nc.gpsimd.dma_start(
                            agin_v[:, :, 34:40], scores_v[:, :, 34:40])

            def body():
                if warm_exp:
                    # warm the ACT Exp table while the PE pipeline fills:
                    # the implicit table load (~1.3us) otherwise lands
                    # right before the tail exp, on the critical path
                    warm = mpool.tile([1, 1], f32, tag="warm", name="warm")
                    nc.scalar.activation(warm[:], warm[:], Act.Exp,
                                         bias=0.0, scale=0.0)
                tail_c = {}

                def load_tail_consts():
                    # ACT hwdge queue: a 1.1 MiB load on the SP queue here
                    # would stall the weight stream behind it
                    teng = nc.scalar if tailc_act else nc.sync
                    tail_c["xpwo"] = tcpool.tile(
                        [128, 8, N_CTX], f16, tag="xpwo", name="xpwo")
                    tail_c["mask"] = tcpool.tile(
                        [RSH, N_CTX], f32, tag="mask", name="mask")
                    teng.dma_start(tail_c["xpwo"][:], xpwo_t[:])
                    teng.dma_start(tail_c["mask"][:], mask_t[:])

                # ---- stage A ---------------------------------------------
                stage_a(load_tail_consts)
                xpwo_sb = tail_c["xpwo"]
                mask_sb = tail_c["mask"]

                # ---- gather remaining columns + AllToAll ------------------
                if gather3 and gather4:
                    # final 4-column chunk on the ACT hwdge queue: skips
                    # the Pool SWDGE descriptor-gen on the critical path
                    (nc.scalar if last_gather_act else nc.gpsimd).dma_start(
                        agin_v[:, :, 30:34], scores_v[:, :, 30:34])
                elif gather3:
                    nc.gpsimd.dma_start(
                        agin_v[:, :, 24:40], scores_v[:, :, 24:40])
                else:
                    nc.gpsimd.dma_start(agin_v[:], scores_v[:])
                if use_collective:
                    nc.gpsimd.collective_compute(
                        "AllToAll",
                        mybir.AluOpType.bypass,
                        replica_groups=[list(range(NCORES))],
                        ins=[agin[:].opt()],
                        outs=[agout[:].opt()],
                    )
                    coll_out = agout
                else:
                    coll_out = agin
                sfull = mpool.tile([RSH, N_CTX], f32, tag="sfull",
                                   name="sfull")
                (nc.scalar if sfull_act else nc.gpsimd).dma_start(
                    sfull[:].rearrange("i (r k) -> i r k", r=NCORES),
                    coll_out[:].rearrange("(r i) k -> i r k", r=NCORES),
                )

                # ---- masked softmax over the 64 rows ----------------------
                nsm = mpool.tile([RSH, N_CTX], f32, tag="sm", name="sm")
                negm = mpool.tile([RSH, 1], f32, tag="negm", name="negm")
                esb = mpool.tile([RSH, N_CTX], f16, tag="esb", name="esb")
                den = mpool.tile([RSH, 1], f32, tag="den", name="den")
                nc.vector.tensor_tensor(
                    out=nsm[:], in0=sfull[:], in1=mask_sb[:],
                    op=Alu.subtract)
                nc.vector.reduce_max(
                    negm[:], nsm[:], axis=mybir.AxisListType.X,
                    negate=True)
                nc.scalar.activation(
                    esb[:], nsm[:], Act.Exp, bias=negm[:], scale=1.0,
                    accum_out=den[:])
                rden = mpool.tile([RSH, 1], f32, tag="rden", name="rden")
                nc.vector.reciprocal(rden[:], den[:])

                # ---- A^T: [64, 512] -> 4x [128, 64] -----------------------
                at_sb = []
                if pe_transpose:
                    pt = ppB.tile([128, 4, RSH], f16, tag="pt", name="pt")
                    for kt in range(4):
                        nc.tensor.transpose(
                            pt[:, kt, :], esb[:, kt * 128:(kt + 1) * 128],
                            ident_sb[:])
                    for kt in range(4):
                        at = mpool.tile([128, RSH], f16, tag=f"at{kt}",
                                        name=f"at{kt}")
                        nc.scalar.copy(at[:], pt[:, kt, :])
                        at_sb.append(at)
                else:
                    for kt in range(4):
                        at = mpool.tile([128, RSH], f16, tag=f"at{kt}",
                                        name=f"at{kt}")
                        nc.scalar.dma_start_transpose(
                            at[:], esb[:, kt * 128:(kt + 1) * 128])
                        at_sb.append(at)

                # ---- O^T = X^T @ A^T -------------------------------------
                ot_sb = []
                for et in range(4):
                    op = ppA.tile([128, RSH], f32, tag="yp", name="op")
                    for kt in range(4):
                        nc.tensor.matmul(
                            op[:, 0:RSH],
                            lhsT=xpwo_sb[:, kt, et * 128:(et + 1) * 128],
                            rhs=at_sb[kt][:],
                            start=(kt == 0),
                            stop=(kt == 3),
                        )
                    ot = mpool.tile([128, RSH], f16, tag=f"ot{et}",
                                    name=f"ot{et}")
                    nc.scalar.copy(ot[:], op[:, 0:RSH])
                    ot_sb.append(ot)

                # ---- Y = O @ W_out^T -------------------------------------
                ypz = ppB.tile([128, 512], f32, tag="pt", name="ypz")
                for et in range(4):
                    nc.tensor.matmul(
                        ypz[0:RSH, :],
                        lhsT=ot_sb[et][:],
                        rhs=xpwo_sb[:, 4 + et, :],
                        start=(et == 0),
                        stop=(et == 3),
                    )
                y_sb = mpool.tile([RSH, D], f32, tag="y_sb", name="y_sb")
                nc.scalar.mul(y_sb[:], ypz[0:RSH, :], rden[:])
                nc.scalar.dma_start(out_t[:], y_sb[:])

            if timing_loop == -1:
                tmp = nc.alloc_registers("niter_reg", mybir.ALL_ENGINES)
                nc.regs_load(tmp, niter_t[0:1, 0:1])
                nval = nc.snap(tmp, donate=True, min_val=0, max_val=1024)
                with tc.For_i(0, nval, 1):
                    body()
            elif timing_loop:
                with tc.For_i(0, timing_loop, 1):
                    body()
            else:
                body()

    nc.compile()
    return nc


def _build_v3(timing_loop=0, use_collective=True, num_devices=NCORES,
              wbufs=6, zbufs=3, n_direct=10, n_pool=20, warm_exp=True,
              st_groups=2, lag_pairs=1, sum_pool=5):
    """Phase-2: W-stationary stage A with exact causal streaming.

    Per local column kk (global k = 8*kk + m), only rows i >= 8*kk are
    computed (nk = 512 - 8*kk, SPMD-uniform).  The U' pack per kk is 10
    [128,128] blocks (upper block-triangle); each is the matmul lhsT and
    X^T streams as rhs: Z[et][e,i] = sum_d U'[d,e] x[i,d], nk columns per
    matmul -> 10*nk PE cycles/kk vs 1280*ceil(nk/128) row-tile cycles in
    the row-stationary layout (166k vs 205k cycles/core).

    scores^T assembly: P = Z (.) X^T reduced over e via an accumulating
    "ones-column" matmul: lhsT = ones32[kk%32] (ones in column kk%32)
    lands the row sum at PSUM partition kk of a [64, 512] score bank, nk
    cycles per kk.  The e-block sum over 4 blocks happens on DVE f16 adds
    beforehand; the PSUM->f16 copy runs on ACT (or the (.) on Pool) for a
    tunable share of columns to balance the three vector engines.
    """
    import concourse.mybir as mybir
    import concourse.tile as tile
    from concourse import bacc

    f32 = mybir.dt.float32
    f16 = mybir.dt.float16
    Alu = mybir.AluOpType
    Act = mybir.ActivationFunctionType

    nc = bacc.Bacc(
        "TRN2", target_bir_lowering=False, debug=False,
        enable_asserts=False, num_devices=num_devices,
    )

    xpwo_t = nc.dram_tensor("xpwo", [128, 8, D], f16,
                            kind="ExternalInput").ap()
    xtp_t = nc.dram_tensor("xtp", [128, 4, N_CTX], f16,
                           kind="ExternalInput").ap()
    wbi_t = nc.dram_tensor("wbi3", [KSH // 2, 128, 2 * PACKW], f16,
                           kind="ExternalInput").ap()
    mask_t = nc.dram_tensor("mask", [RSH, N_CTX], f32,
                            kind="ExternalInput").ap()
    ident_t = nc.dram_tensor("ident", [64, 64], f16,
                             kind="ExternalInput").ap()
    ident32_t = nc.dram_tensor("ident32", [64, 64], f32,
                               kind="ExternalInput").ap()
    ones32_t = nc.dram_tensor("ones32", [128, 32, 32], f16,
                              kind="ExternalInput").ap()
    niter_t = (nc.dram_tensor("niter", [1, 1], mybir.dt.int32,
                              kind="ExternalInput").ap()
               if timing_loop == -1 else None)
    out_t = nc.dram_tensor("out", [RSH, D], f32, kind="ExternalOutput").ap()

    with tile.TileContext(nc) as tc:
        with (
            tc.tile_pool(name="const", bufs=1) as cpool,
            tc.tile_pool(name="tailc", bufs=2) as tcpool,
            tc.tile_pool(name="wstream", bufs=wbufs) as wpool,
            tc.tile_pool(name="scratch", bufs=4) as spool,
            tc.tile_pool(name="scratch2", bufs=4) as spool2,
            tc.tile_pool(name="small", bufs=1) as mpool,
            tc.tile_pool(name="psZ", bufs=zbufs, space="PSUM") as ppZ,
            tc.tile_pool(name="psS", bufs=2, space="PSUM") as ppS,
            tc.tile_pool(name="dram", bufs=1, space="DRAM") as dpool,
        ):
            # ---- residents ------------------------------------------------
            xtp_sb = cpool.tile([128, 4, N_CTX], f16, tag="xtp", name="xtp")
            nc.sync.dma_start(xtp_sb[:], xtp_t[:])
            ones_sb = cpool.tile([128, 32, 32], f16, tag="o32", name="o32")
            ident_sb = cpool.tile([64, 64], f16, tag="ident", name="ident")
            id32_sb = cpool.tile([64, 64], f32, tag="id32", name="id32")

            def load_consts():
                nc.gpsimd.dma_start(ones_sb[:], ones32_t[:])
                nc.gpsimd.dma_start(ident_sb[:], ident_t[:])
                nc.gpsimd.dma_start(id32_sb[:], ident32_t[:])
            scores_sb = cpool.tile([128, 4 * KSH], f32, tag="sc", name="sc")
            zlhs_sb = cpool.tile([128, 32], f16, tag="zl", name="zl")
            nc.gpsimd.memset(zlhs_sb[:], 0.0)
            agin = dpool.tile([N_CTX, KSH], f32, tag="agin")
            agout = dpool.tile([N_CTX, KSH], f32, tag="agout")
            agin_v = agin[:].rearrange("(t p) k -> p t k", p=128)
            scores_v = scores_sb[:].rearrange("p (t k) -> p t k", t=4)

            def load_wk_pair(j, split=1):
                wk = wpool.tile([128, 2 * PACKW], f16, tag="wk", name="wk")
                if split == 1:
                    nc.sync.dma_start(wk[:], wbi_t[j])
                else:
                    nc.sync.dma_start(wk[:, 0:PACKW], wbi_t[j][:, 0:PACKW])
                    nc.sync.dma_start(wk[:, PACKW:], wbi_t[j][:, PACKW:])
                return wk

            # (et, dt) block order within a half-pack: et asc, dt asc
            BLK = {}
            _i = 0
            for et in range(4):
                for dt in range(et + 1):
                    BLK[(et, dt)] = _i
                    _i += 1

            # vector-chain routing per kk: "direct" (DVE stt from PSUM),
            # "actdve" (ACT copy f16 -> DVE (.) + adds), "actpool" (ACT
            # copy -> Pool (.) -> DVE adds)
            def route_of(kk):
                r = (kk * 7) % 16
                if r < n_direct:
                    return "direct"
                if r < n_direct + n_pool:
                    return "actpool"
                return "actdve"

            def body(st_banks):
                if warm_exp:
                    warm = mpool.tile([1, 1], f32, tag="warm", name="warm")
                    nc.scalar.activation(warm[:], warm[:], Act.Exp,
                                         bias=0.0, scale=0.0)
                tail_c = {}

                def load_tail_consts():
                    tail_c["xpwo"] = tcpool.tile(
                        [128, 8, N_CTX], f16, tag="xpwo", name="xpwo")
                    tail_c["mask"] = tcpool.tile(
                        [RSH, N_CTX], f32, tag="mask", name="mask")
                    nc.scalar.dma_start(tail_c["xpwo"][:], xpwo_t[:])
                    nc.scalar.dma_start(tail_c["mask"][:], mask_t[:])

                # ---- stage A: Z = U'^T-blocks @ X^T, scores^T rows -------
                # Z is produced in n-chunks of <=256 columns: one
                # [128, 4(et), 256] f32 tile = 2 PSUM banks, zbufs in
                # flight, so the vector chain for chunk c runs while the
                # PE fills chunk c+1/c+2 (one-pair emission lag).
                # st banks are zero-filled once via a zero-weights matmul
                # so every st-mm can accumulate (start=False) regardless
                # of its column range (group 1 sees kk descending).
                for g in range(st_groups):
                    nc.tensor.matmul(
                        st_banks[g][0:32, :],
                        lhsT=zlhs_sb[:], rhs=xtp_sb[:, 0, :],
                        start=True, stop=False,
                        skip_group_check=True,
                    )
                pend = []          # deferred vector+st work, one pair lag
                count_in = [0] * st_groups
                CHUNKS_G = [0] * st_groups
                for kk in range(KSH):
                    CHUNKS_G[kk * st_groups // KSH] += (
                        1 if N_CTX - 8 * kk <= 256 else 2)

                def emit_zchunk(wk, half, kk, c):
                    base_blk = half * 10
                    n0 = 8 * kk + 256 * c
                    cw = min(256, N_CTX - n0)
                    zc = ppZ.tile([128, 4, 256], f32, tag="z", name="zc")
                    for et in range(4):
                        for dt in range(et + 1):
                            blk = (base_blk + BLK[(et, dt)]) * 128
                            nc.tensor.matmul(
                                zc[:, et, 0:cw],
                                lhsT=wk[:, blk:blk + 128],
                                rhs=xtp_sb[:, dt, n0:n0 + cw],
                                start=(dt == 0),
                                stop=(dt == et),
                                skip_group_check=True,
                            )
                    return zc, n0, cw

                chunk_i = [0]

                def emit_vec_st(kk, zc, n0, cw, last_chunk):
                    g = st_group(kk)
                    route = route_of(chunk_i[0])
                    chunk_i[0] += 1
                    xs = xtp_sb[:, 0:4, n0:n0 + cw]
                    p = spool.tile([128, 4, 256], f16, tag="p16",
                                   name="p16")
                    if route == "direct":
                        nc.vector.scalar_tensor_tensor(
                            out=p[:, :, 0:cw], in0=zc[:, :, 0:cw],
                            scalar=1.0, in1=xs,
                            op0=Alu.mult, op1=Alu.mult)
                    else:
                        c16 = spool2.tile([128, 4, 256], f16, tag="c16",
                                          name="c16")
                        nc.scalar.copy(c16[:, :, 0:cw], zc[:, :, 0:cw])
                        if route == "actpool":
                            nc.gpsimd.tensor_tensor(
                                out=p[:, :, 0:cw], in0=c16[:, :, 0:cw],
                                in1=xs, op=Alu.mult)
                        else:
                            nc.vector.tensor_tensor(
                                out=p[:, :, 0:cw], in0=c16[:, :, 0:cw],
                                in1=xs, op=Alu.mult)
                    sum_eng = (nc.gpsimd if (chunk_i[0] * 11) % 16 < sum_pool
                               else nc.vector)
                    e2 = spool.tile([128, 2, 256], f16, tag="e2", name="e2")
                    sum_eng.tensor_tensor(
                        out=e2[:, :, 0:cw], in0=p[:, 0:2, 0:cw],
                        in1=p[:, 2:4, 0:cw], op=Alu.add)
                    pacc = spool.tile([128, 256], f16, tag="pacc",
                                      name="pacc")
                    sum_eng.tensor_tensor(
                        out=pacc[:, 0:cw], in0=e2[:, 0, 0:cw],
                        in1=e2[:, 1, 0:cw], op=Alu.add)
                    count_in[g] += 1
                    nc.tensor.matmul(
                        st_banks[g][0:32, n0:n0 + cw],
                        lhsT=ones_sb[:, kk % 32, :],
                        rhs=pacc[:, 0:cw],
                        start=False, stop=(count_in[g] == CHUNKS_G[g]),
                        skip_group_check=True,
                    )

                def st_group(kk):
                    return kk * st_groups // KSH

                def flush_pend():
                    while pend:
                        fn = pend.pop(0)
                        fn()

                for j in range(KSH // 2):
                    wk = load_wk_pair(j, split=(2 if j < 2 else 1))
                    if j == 0:
                        load_consts()
                    if j == 10:
                        load_tail_consts()
                    this_pair = []
                    for half, kk in enumerate((j, KSH - 1 - j)):
                        nk = N_CTX - 8 * kk
                        for c in range(1 if nk <= 256 else 2):
                            zc, n0, cw = emit_zchunk(wk, half, kk, c)
                            this_pair.append((kk, zc, n0, cw,
                                              c == (0 if nk <= 256 else 1)))
                    # deferred vector + st work from the previous pair
                    flush_pend()

                    def mk(args):
                        def fn():
                            emit_vec_st(*args)
                        return fn
                    for args in this_pair:
                        pend.append(mk(args))
                flush_pend()

                # ---- scores^T -> scores (PE transposes), gather ----------
                stT = mpool.tile([64, N_CTX], f32, tag="stT", name="stT")
                nc.scalar.copy(stT[0:32, :], st_banks[0][0:32, :])
                nc.scalar.copy(stT[32:64, :], st_banks[1][0:32, :])
                # transpose [64kk, 512i] -> 4x [128i, 64kk]
                tp = ppZ.tile([128, 2, N_CTX], f32, tag="z", name="tp")
                for nt in range(4):
                    nc.tensor.matmul(
                        tp[:, nt % 2, nt // 2 * 64:nt // 2 * 64 + 64],
                        lhsT=stT[:, nt * 128:(nt + 1) * 128],
                        rhs=id32_sb[:],
                        is_transpose=True,
                        skip_group_check=True,
                    )
                for nt in range(4):
                    nc.scalar.copy(
                        scores_sb[:, nt * KSH:(nt + 1) * KSH],
                        tp[:, nt % 2, nt // 2 * 64:nt // 2 * 64 + 64])
                    nc.gpsimd.dma_start(agin_v[:, nt, :],
                                        scores_v[:, nt, :])

                xpwo_sb = tail_c["xpwo"]
                mask_sb = tail_c["mask"]
                if use_collective:
                    nc.gpsimd.collective_compute(
                        "AllToAll",
                        mybir.AluOpType.bypass,
                        replica_groups=[list(range(NCORES))],
                        ins=[agin[:].opt()],
                        outs=[agout[:].opt()],
                    )
                    coll_out = agout
                else:
                    coll_out = agin
                sfull = mpool.tile([RSH, N_CTX], f32, tag="sfull",
                                   name="sfull")
                nc.scalar.dma_start(
                    sfull[:].rearrange("i (r k) -> i r k", r=NCORES),
                    coll_out[:].rearrange("(r i) k -> i r k", r=NCORES),
                )

                # ---- masked softmax + attn tail (as _build_v2) -----------
                nsm = mpool.tile([RSH, N_CTX], f32, tag="sm", name="sm")
                negm = mpool.tile([RSH, 1], f32, tag="negm", name="negm")
                esb = mpool.tile([RSH, N_CTX], f16, tag="esb", name="esb")
                den = mpool.tile([RSH, 1], f32, tag="den", name="den")
                nc.vector.tensor_tensor(
                    out=nsm[:], in0=sfull[:], in1=mask_sb[:],
                    op=Alu.subtract)
                nc.vector.reduce_max(
                    negm[:], nsm[:], axis=mybir.AxisListType.X,
                    negate=True)
                nc.scalar.activation(
                    esb[:], nsm[:], Act.Exp, bias=negm[:], scale=1.0,
                    accum_out=den[:])
                rden = mpool.tile([RSH, 1], f32, tag="rden", name="rden")
                nc.vector.reciprocal(rden[:], den[:])

                pt = ppS.tile([128, 4, RSH], f16, tag="st", name="ptT")
                at_sb = []
                for kt in range(4):
                    nc.tensor.matmul(
                        pt[:, kt, :],
                        lhsT=esb[:, kt * 128:(kt + 1) * 128],
                        rhs=ident_sb[:],
                        is_transpose=True,
                        skip_group_check=True,
                    )
                for kt in range(4):
                    at = mpool.tile([128, RSH], f16, tag=f"at{kt}",
                                    name=f"at{kt}")
                    nc.scalar.copy(at[:], pt[:, kt, :])
                    at_sb.append(at)

                ot_sb = []
                for et in range(4):
                    op = ppZ.tile([128, 2, N_CTX], f32, tag="z", name="op")
                    for kt in range(4):
                        nc.tensor.matmul(
                            op[:, 0, 0:RSH],
                            lhsT=xpwo_sb[:, kt, et * 128:(et + 1) * 128],
                            rhs=at_sb[kt][:],
                            start=(kt == 0),
                            stop=(kt == 3),
                        )
                    ot = mpool.tile([128, RSH], f16, tag=f"ot{et}",
                                    name=f"ot{et}")
                    nc.scalar.copy(ot[:], op[:, 0, 0:RSH])
                    ot_sb.append(ot)

                ypz = ppZ.tile([128, 2, N_CTX], f32, tag="z", name="ypz")
                for et in range(4):
                    nc.tensor.matmul(
                        ypz[0:RSH, 0, :],
                        lhsT=ot_sb[et][:],
                        rhs=xpwo_sb[:, 4 + et, :],
                        start=(et == 0),
                        stop=(et == 3),
                    )
                y_sb = mpool.tile([RSH, D], f32, tag="y_sb", name="y_sb")
                nc.scalar.mul(y_sb[:], ypz[0:RSH, 0, :], rden[:])
                nc.scalar.dma_start(out_t[:], y_sb[:])

            def alloc_st():
                return [ppS.tile([32, N_CTX], f32, tag="st",
                                 name=f"st{g}") for g in range(st_groups)]

            if timing_loop == -1:
                tmp = nc.alloc_registers("niter_reg", mybir.ALL_ENGINES)
                nc.regs_load(tmp, niter_t[0:1, 0:1])
                nval = nc.snap(tmp, donate=True, min_val=0, max_val=1024)
                with tc.For_i(0, nval, 1):
                    body(alloc_st())
            elif timing_loop:
                with tc.For_i(0, timing_loop, 1):
                    body(alloc_st())
            else:
                body(alloc_st())

    nc.compile()
    return nc


def _pack_blocks(Wm):
    """[KSH, 512, 512] fp32 -> [KSH, 128, PACKW] fp16 block pack for the
    W-stationary stage A: per k, the 10 upper-block-triangle [128,128]
    blocks of U' in (et, dt) order (et asc, dt asc within et)."""
    U = np.triu(Wm + Wm.transpose(0, 2, 1), 1)
    idx = np.arange(D)
    U[:, idx, idx] = Wm[:, idx, idx]
    pack = np.empty((Wm.shape[0], 128, PACKW), np.float16)
    i = 0
    for et in range(4):
        for dt in range(et + 1):
            pack[:, :, i * 128:(i + 1) * 128] = \
                U[:, 128 * dt:128 * dt + 128, 128 * et:128 * et + 128]
            i += 1
    return pack


def _pack_upper(Wm):
    """[KSH, 512, 512] fp32 -> [KSH, 128, PACKW] fp16 upper-tri pack.

    U' = triu(W + W^T, 1) + diag(W); block dt holds U'[128dt+p, 128dt:512].
    """
    U = np.triu(Wm + Wm.transpose(0, 2, 1), 1)
    idx = np.arange(D)
    U[:, idx, idx] = Wm[:, idx, idx]
    pack = np.empty((KSH, 128, PACKW), np.float16)
    for dt in range(4):
        lo = 128 * dt
        pack[:, :, OFFS[dt]:OFFS[dt] + SPANS[dt]] = U[:, lo:lo + 128, lo:D]
    return pack


def _make_in_maps(x, W_bi, W_out, stage_a="causal"):
    x = np.ascontiguousarray(np.asarray(x, dtype=np.float32))
    W_bi = np.asarray(W_bi, dtype=np.float32)
    W_out = np.asarray(W_out, dtype=np.float32)
    # x row-major packed [p, nt, d] = x[128*nt + p, d]
    xpk = np.ascontiguousarray(x.reshape(4, 128, D).transpose(1, 0, 2))
    x16k = xpk.astype(np.float16)
    # xtp[p, dt, n] = x[n, 128*dt + p]
    xtp16 = np.ascontiguousarray(
        x.T.reshape(4, 128, N_CTX).transpose(1, 0, 2)).astype(np.float16)
    # interleaved k-sharding: core m owns global columns k = 8*kk + m.
    # After the AllToAll gather, score column position p = r*64 + kk
    # holds global k = 8*kk + r, so X rows and the causal mask are
    # permuted to match.
    perm = np.array([8 * (p % KSH) + p // KSH for p in range(N_CTX)])
    xperm = x[perm]
    woutt = W_out.T
    # xpwo[p, 0:4, :] = xperm blocks, xpwo[p, 4+et, :] = W_out^T blocks
    xpwo = np.empty((128, 8, D), np.float16)
    xpwo[:, 0:4, :] = xperm.reshape(4, 128, D).transpose(1, 0, 2)
    xpwo[:, 4:8, :] = woutt.reshape(4, 128, D).transpose(1, 0, 2)
    kcol = perm[None, :]                       # global k at position p
    in_maps = []
    for m in range(NCORES):
        pack = _pack_upper(np.ascontiguousarray(W_bi[m::NCORES]))
        # pair layout: [j] = concat(pack[j], pack[63-j]) along the free dim
        pairs = np.concatenate([pack[:KSH // 2], pack[:KSH // 2 - 1:-1]],
                               axis=2)
        rows = np.arange(m * RSH, (m + 1) * RSH)[:, None]
        # negated mask: 0 where allowed, +1e30 where masked
        mask = np.where(kcol <= rows, 0.0, -NEG_INF).astype(np.float32)
        pack3 = _pack_blocks(np.ascontiguousarray(W_bi[m::NCORES]))
        pairs3 = np.concatenate([pack3[:KSH // 2], pack3[:KSH // 2 - 1:-1]],
                                axis=2)
        ones32 = np.zeros((128, 32, 32), np.float16)
        for c in range(32):
            ones32[:, c, c] = 1.0
        in_maps.append({
            "x": xpk,
            "x16": x16k,
            "xpwo": xpwo,
            "xtp": xtp16,
            "wbi": np.ascontiguousarray(pairs),
            "wbi3": np.ascontiguousarray(pairs3),
            "mask": np.ascontiguousarray(mask),
            "ident": np.eye(64, dtype=np.float16),
            "ident32": np.eye(64, dtype=np.float32),
            "ones32": ones32,
        })
    return in_maps


def kernel(x, W_bi, W_out):
    global _nc_cache
    import time as _time
    from concourse.bass_utils import run_bass_kernel_spmd

    if _nc_cache is None:
        _nc_cache = _build_v2(**V2_KW)
    nc = _nc_cache
    in_maps = _make_in_maps(x, W_bi, W_out, stage_a=STAGE_A)
    last_exc = None
    for attempt in range(3):
        try:
            res = run_bass_kernel_spmd(nc, in_maps, core_ids=list(range(NCORES)),
                                       trace=False)
            break
        except Exception as e:  # transient NRT/axon wedges recover on retry
            last_exc = e
            _time.sleep(5.0 * (attempt + 1))
    else:
        raise last_exc
    out = np.concatenate([res.results[m]["out"] for m in range(NCORES)], axis=0)
    return np.ascontiguousarray(out, dtype=np.float32)

